# revision 1
# baseline (speedup 1.0000x reference)
"""Trainium2 Bass kernel for nn_MjCambrianOptics (depth-invariant PSF + FFT blur).

Self-contained; hardcoded shapes. Two SPMD launches on 8 NeuronCores.

Launch A (PSF): the reference's H enters via @ (batched matmul), so ifft2's
column transform cancels fft2's (W @ Winv = I):
    u3 = S- (Winv @ Hs @ (W S) @ u2) S-^T
E = Winv @ Hs @ W1s @ u2supp is only 3 matmuls with a 511-wide aperture
support. Phases fl(k*r), fl((k*md)*q) are reproduced bit-exactly on device
(IEEE DVE ops, Cody-Waite reduction, ACT Sin on [-pi,pi]); md and the r
support block (needs IEEE sqrt, ACT Sqrt is not IEEE) come from the host.
Sharding: cores 0-5 = (channel, 256-col half); cores 6,7 dummy.

Host between launches: permutation-assemble psf, sum S, flip; pure gather.

Launch B (CONV): out_c = Re(WL @ (Fimg .* Fpsf)[0:769,:] @ WR^T) at P=1536
(circular size >= 1534 is alias-free for the needed crop), Hermitian row
truncation. 24 units = 3 channels x 8 freq-row blocks; 3 units/core; each
unit emits a 511x511 partial-sum output; host sums per channel, scales,
clips (bilinear partial-sum unshard).

Matmul precision: f32r (TF32-like 12-bit) everywhere = ~1e-4/stage noise.
"""
import numpy as np

import concourse.bacc as bacc
import concourse.mybir as mybir
import concourse.tile as tile
from concourse.bass_utils import run_bass_kernel_spmd

F32 = mybir.dt.float32
F32R = mybir.dt.float32r
AF = mybir.ActivationFunctionType
ALU = mybir.AluOpType

MX = 1023
RES = 511
S0 = 256
NS = 511
SENSOR = 0.01
APERTURE = 0.5
WAVELENGTHS = np.array([610e-9, 530e-9, 470e-9], dtype=np.float32)
P = 1536
NCORE = 8

PI = np.float32(np.pi)
TWO_PI = np.float64(2.0) * np.pi
C_RND = float(np.float32(1.5 * 2.0 ** 23))

KC_S = [(0, 128), (128, 128), (256, 128), (384, 127)]        # 511 rows
KC_M = [(i * 128, 128) for i in range(7)] + [(896, 127)]     # 1023 rows
KC_P = [(i * 128, 128) for i in range(12)]                   # 1536 rows


def _r32c(x):
    """Round ndarray to f32r (12-bit significand), RNE — matches tensor_copy."""
    f = np.ascontiguousarray(x, np.float32)
    b = f.view(np.uint32).astype(np.uint64)
    low = b & 0xFFF
    b2 = b & ~np.uint64(0xFFF)
    up = (low > 0x800) | ((low == 0x800) & (((b2 >> 12) & 1) == 1))
    b2 = b2 + np.where(up, np.uint64(0x1000), np.uint64(0))
    return b2.astype(np.uint32).view(np.float32).reshape(f.shape)


def _splitb(x64, keep):
    f = np.float32(x64)
    mask = np.uint32(0xFFFFFFFF ^ ((1 << (24 - keep)) - 1))
    bits = np.uint32(int(f.view(np.uint32)) & int(mask))
    return bits.view(np.float32)


P1 = _splitb(TWO_PI, 11)                       # 11-bit
P2 = _splitb(TWO_PI - np.float64(P1), 10)      # 10-bit
P3 = np.float32(TWO_PI - np.float64(P1) - np.float64(P2))
P1H = np.float32(np.float64(P1) * 256.0)
P2H = np.float32(np.float64(P2) * 256.0)

_CONSTS = {}


def _consts():
    if _CONSTS:
        return _CONSTS
    dx = SENSOR / MX
    Lx = dx * MX
    x1 = np.linspace(-Lx / 2, Lx / 2, MX, dtype=np.float32)
    X1, Y1 = np.meshgrid(x1, x1, indexing="ij")
    fx = np.linspace(-1.0 / (2 * dx), 1.0 / (2 * dx), MX, dtype=np.float32)
    FX, FY = np.meshgrid(fx, fx, indexing="ij")
    ar = (Lx / 2.0) * APERTURE
    A = (np.sqrt(X1 ** 2 + Y1 ** 2) / np.float32(ar + 1e-7) <= 1.0).astype(np.float32)
    lam = WAVELENGTHS
    k_arr = (np.float32(2.0) * np.float32(np.pi) / lam).astype(np.float32)
    q = np.empty((3, MX, MX), np.float32)
    for c in range(3):
        a_ = (lam[c] * FX).astype(np.float32)
        b_ = (lam[c] * FY).astype(np.float32)
        s_ = ((np.float32(1.0) - (a_ * a_).astype(np.float32)).astype(np.float32)
              - (b_ * b_).astype(np.float32)).astype(np.float32)
        q[c] = np.sqrt(s_).astype(np.float32)
    R2 = ((X1 * X1).astype(np.float32) + (Y1 * Y1).astype(np.float32)).astype(np.float32)

    jk = np.arange(MX)
    W = np.exp(-2j * np.pi * np.outer(jk, jk) / MX)
    Winv = np.conj(W) / MX
    perm_s = (jk - MX // 2) % MX
    perm_si = (jk + MX // 2) % MX
    WS_s = W[:, perm_si][:, S0:S0 + NS]        # [1023 x 511]
    qs = q[:, perm_s][:, :, perm_s]            # Hs = exp(i t1 qs)

    jP = np.arange(P)
    Wp = np.exp(-2j * np.pi * np.outer(jP, jP) / P)
    Wg = Wp[:, :MX]
    Winvp = np.conj(Wp) / P
    selr = 767 + np.arange(RES)
    WL = Winvp[selr, :769].copy()
    WL[:, 1:768] *= 2.0
    WLz = np.zeros((RES, 1024), np.complex128)
    WLz[:, :769] = WL
    WR = Winvp[selr, :]

    C = {}
    C["k_arr"] = k_arr
    C["R2supp"] = R2[S0:S0 + NS, S0:S0 + NS]
    C["A_supp"] = A[S0:S0 + NS, S0:S0 + NS]
    C["qs"] = qs
    C["perm_s"] = perm_s
    C["perm_si"] = perm_si
    C["w1sT_re"] = _r32c(np.real(WS_s).T)      # [511 x 1023] lhsT step1
    C["w1sT_im"] = _r32c(np.imag(WS_s).T)
    C["winv_re"] = _r32c(np.real(Winv))        # symmetric
    C["winv_im"] = _r32c(np.imag(Winv))
    C["WgT_re"] = _r32c(np.real(Wg).T)         # [1023 x 1536]
    C["WgT_im"] = _r32c(np.imag(Wg).T)
    C["Wg"] = Wg
    wrt_pad = np.zeros((P, 512), np.float64)
    wrt_pad[:, :RES] = np.real(WR).T
    C["WRT_re"] = _r32c(wrt_pad)               # [1536 x 512] (padded)
    wrt_pad2 = np.zeros((P, 512), np.float64)
    wrt_pad2[:, :RES] = np.imag(WR).T
    C["WRT_im"] = _r32c(wrt_pad2)
    C["WRT_imN"] = _r32c(-wrt_pad2)
    C["WLz"] = WLz
    C["ident"] = np.eye(128, dtype=np.float32)
    _CONSTS.update(C)
    return _CONSTS


def _cascade_scalars(theta_lo, theta_hi):
    n0 = int(np.floor((0.5 * (theta_lo + theta_hi)) / TWO_PI + 0.5))
    n0h = (n0 // 4096) * 4096
    n0l = n0 - n0h
    A0 = np.float32(np.float64(n0h) * np.float64(P1))
    B0 = np.float32(np.float64(n0l) * np.float64(P1))
    C0 = np.float32(np.float64(n0h) * np.float64(P2))
    D0 = np.float32(np.float64(n0l) * np.float64(P2))
    E0 = np.float32(np.float64(n0) * np.float64(P3))
    assert np.float64(A0) == n0h * np.float64(P1)
    assert np.float64(B0) == n0l * np.float64(P1)
    assert np.float64(C0) == n0h * np.float64(P2)
    assert np.float64(D0) == n0l * np.float64(P2)
    return A0, B0, C0, D0, E0


def host_trig_model(base, t, casc=None):
    """numpy mirror of device phase pipeline -> (cos, sin) of fl(t*base)."""
    th = (np.float32(t) * np.asarray(base, np.float32)).astype(np.float32)
    CR = np.float32(C_RND)
    f = ((th * np.float32(1.0 / TWO_PI)).astype(np.float32) + CR).astype(np.float32)
    n = (f - CR).astype(np.float32)
    g = ((n * np.float32(1.0 / 256.0)).astype(np.float32) + CR).astype(np.float32)
    nh = (g - CR).astype(np.float32)
    def cw(x, k, c1, c2, c3):
        y = (x - (k * np.float32(c1)).astype(np.float32)).astype(np.float32)
        y = (y - (k * np.float32(c2)).astype(np.float32)).astype(np.float32)
        return (y - (k * np.float32(c3)).astype(np.float32)).astype(np.float32)
    nl = cw(n, nh, np.float32(256.0), 0.0, 0.0)
    y = cw(th, nh, P1H, P2H, 0.0)
    y = cw(y, nl, P1, P2, 0.0)
    y = cw(y, n, P3, 0.0, 0.0)
    TP = np.float32(2 * np.pi)
    yw = (y + TP * ((y < -PI).astype(np.float32) - (y > PI).astype(np.float32))).astype(np.float32)
    yc = (y + np.float32(PI / 2)).astype(np.float32)
    yc = (yc + TP * ((yc < -PI).astype(np.float32) - (yc > PI).astype(np.float32))).astype(np.float32)
    return np.sin(yc.astype(np.float64)), np.sin(yw.astype(np.float64)), th


# ---------------------------------------------------------------------------
# Launch A
# ---------------------------------------------------------------------------
_NCA = {}


def _build_A():
    if "nc" in _NCA:
        return _NCA["nc"]
    nc = bacc.Bacc("TRN2", target_bir_lowering=False, debug=False)
    C = _consts()
    ins = {}
    for nm in ["w1sT_re", "w1sT_im", "winv_re", "winv_im"]:
        ins[nm] = nc.inline_tensor(C[nm], nm).ap().bitcast(F32R)
    ins["qs"] = nc.dram_tensor("qs", [MX, MX], F32, kind="ExternalInput").ap()
    ins["rs"] = nc.dram_tensor("rs", [NS, 256], F32, kind="ExternalInput").ap()
    ins["msk"] = nc.dram_tensor("msk", [NS, 256], F32, kind="ExternalInput").ap()
    ins["sc"] = nc.dram_tensor("sc", [128, 16], F32, kind="ExternalInput").ap()
    out_psf = nc.dram_tensor("psf", [1024, 256], F32, kind="ExternalOutput").ap()

    with tile.TileContext(nc) as tc:
        with (
            tc.tile_pool(name="cst", bufs=1) as cp,
            tc.tile_pool(name="trg", bufs=1) as tg,
            tc.tile_pool(name="stt", bufs=1) as sp,
            tc.tile_pool(name="wts", bufs=3) as wp,
            tc.tile_pool(name="psA", bufs=2, space="PSUM") as pp,
        ):
            scal = cp.tile([128, 16], F32, tag="scal")
            nc.sync.dma_start(scal[:], ins["sc"][:])

            def trig_pair(dst_cos, dst_sin, base_ap, t_col, casc0, rows, w, mask_ap=None):
                th = tg.tile([128, MX], F32, tag="th")
                nc.vector.tensor_scalar_mul(th[:rows, :w], base_ap, scal[:rows, t_col:t_col + 1])
                f = tg.tile([128, MX], F32, tag="f")
                nc.vector.tensor_scalar(f[:rows, :w], th[:rows, :w],
                                        float(np.float32(1.0 / TWO_PI)), C_RND,
                                        ALU.mult, ALU.add)
                nc.vector.tensor_scalar_sub(f[:rows, :w], f[:rows, :w], C_RND)
                g = tg.tile([128, MX], F32, tag="g")
                nc.vector.tensor_scalar(g[:rows, :w], f[:rows, :w],
                                        float(np.float32(1.0 / 256.0)), C_RND,
                                        ALU.mult, ALU.add)
                nc.vector.tensor_scalar_sub(g[:rows, :w], g[:rows, :w], C_RND)
                nl = tg.tile([128, MX], F32, tag="nl")
                nc.vector.cody_waite_cascade(nl[:rows, :w], f[:rows, :w], g[:rows, :w],
                                             256.0, 0.0, 0.0)
                y = tg.tile([128, MX], F32, tag="y")
                nc.vector.cody_waite_cascade(y[:rows, :w], th[:rows, :w], g[:rows, :w],
                                             float(P1H), float(P2H), 0.0)
                nc.vector.cody_waite_cascade(y[:rows, :w], y[:rows, :w], nl[:rows, :w],
                                             float(P1), float(P2), 0.0)
                nc.vector.cody_waite_cascade(y[:rows, :w], y[:rows, :w], f[:rows, :w],
                                             float(P3), 0.0, 0.0)
                yw = tg.tile([128, MX], F32, tag="yw")
                nc.vector.add_range_wrap(yw[:rows, :w], y[:rows, :w], 0.0, float(PI),
                                         float(np.float32(2 * np.pi)))
                yc = tg.tile([128, MX], F32, tag="yc")
                nc.vector.add_range_wrap(yc[:rows, :w], y[:rows, :w],
                                         float(np.float32(PI / 2)), float(PI),
                                         float(np.float32(2 * np.pi)))
                if mask_ap is None:
                    nc.scalar.activation(dst_sin, yw[:rows, :w], AF.Sin)
                    nc.scalar.activation(dst_cos, yc[:rows, :w], AF.Sin)
                else:
                    sn = tg.tile([128, 256], F32, tag="sn")
                    cn = tg.tile([128, 256], F32, tag="cn")
                    nc.scalar.activation(sn[:rows, :w], yw[:rows, :w], AF.Sin)
                    nc.scalar.activation(cn[:rows, :w], yc[:rows, :w], AF.Sin)
                    nc.vector.tensor_tensor(dst_sin, sn[:rows, :w], mask_ap, ALU.mult)
                    nc.vector.tensor_tensor(dst_cos, cn[:rows, :w], mask_ap, ALU.mult)

            # ---- u2 trig -> stacked [re|im] chunks + negated-im planes ----
            u2_m = sp.tile([128, 4 * 512], F32R, tag="u2m")     # [re|im] per K-chunk
            u2_n = sp.tile([128, 4 * 256], F32R, tag="u2n")     # -im per K-chunk
            for ci, (r0, rn) in enumerate(KC_S):
                rsl = tg.tile([128, 256], F32, tag="rsl")
                nc.sync.dma_start(rsl[:rn], ins["rs"][r0:r0 + rn, :])
                mkl = tg.tile([128, 256], F32, tag="mkl")
                nc.sync.dma_start(mkl[:rn], ins["msk"][r0:r0 + rn, :])
                o = ci * 512
                trig_pair(u2_m[:rn, o:o + 256], u2_m[:rn, o + 256:o + 512],
                          rsl[:rn], 0, 2, rn, 256, mask_ap=mkl[:rn])
                nc.vector.tensor_scalar_mul(u2_n[:rn, ci * 256:(ci + 1) * 256],
                                            u2_m[:rn, o + 256:o + 512].bitcast(F32), -1.0)

            def cmm(acc, lre, lim, m_full, m_re, m_im_neg, ci, last):
                """acc[re|im] += (lre + i lim) @ (m_re + i m_im) via 3 mm."""
                nc.tensor.matmul(acc[:, 0:512], lre, m_full, start=(ci == 0), stop=False)
                nc.tensor.matmul(acc[:, 0:256], lim, m_im_neg, start=False, stop=False)
                nc.tensor.matmul(acc[:, 256:512], lim, m_re, start=False, stop=last)

            # ---- Hs trig chunks (issued early; overlap with step1 matmuls) ----
            hs_cache = {}

            def hs_chunk(ci, r0, rn):
                if ci not in hs_cache:
                    qsl = tg.tile([128, MX], F32, tag="qsl")
                    nc.sync.dma_start(qsl[:rn], ins["qs"][r0:r0 + rn, :])
                    hre = sp.tile([128, MX], F32R, tag=f"hre{ci}")
                    him = sp.tile([128, MX], F32R, tag=f"him{ci}")
                    trig_pair(hre[:rn], him[:rn], qsl[:rn], 1, 7, rn, MX)
                    hs_cache[ci] = (hre, him)
                return hs_cache[ci]

            for ci, (r0, rn) in enumerate(KC_M):
                hs_chunk(ci, r0, rn)

            # ---- step1: X1 = WS_s @ u2 ----
            x1_m = sp.tile([128, 8 * 512], F32R, tag="x1m")
            x1_n = sp.tile([128, 8 * 256], F32R, tag="x1n")
            for mi, (m0, mn) in enumerate(KC_M):
                acc = pp.tile([128, 512], F32, tag="accA")
                for ci, (r0, rn) in enumerate(KC_S):
                    lre = wp.tile([128, 128], F32R, tag="lre")
                    lim = wp.tile([128, 128], F32R, tag="lim")
                    nc.sync.dma_start(lre[:rn, :mn], ins["w1sT_re"][r0:r0 + rn, m0:m0 + mn])
                    nc.sync.dma_start(lim[:rn, :mn], ins["w1sT_im"][r0:r0 + rn, m0:m0 + mn])
                    o = ci * 512
                    cmm(acc[:mn], lre[:rn, :mn], lim[:rn, :mn],
                        u2_m[:rn, o:o + 512], u2_m[:rn, o:o + 256],
                        u2_n[:rn, ci * 256:(ci + 1) * 256], ci, ci == 3)
                o = mi * 512
                nc.vector.tensor_copy(x1_m[:mn, o:o + 512], acc[:mn])
                nc.vector.tensor_scalar_mul(x1_n[:mn, mi * 256:(mi + 1) * 256],
                                            acc[:mn, 256:512], -1.0)

            # ---- step2: X2 = Hs @ X1 ----
            x2_m = sp.tile([128, 8 * 512], F32R, tag="x2m")
            x2_n = sp.tile([128, 8 * 256], F32R, tag="x2n")
            for mi, (m0, mn) in enumerate(KC_M):
                acc = pp.tile([128, 512], F32, tag="accB")
                for ci, (r0, rn) in enumerate(KC_M):
                    hre, him = hs_chunk(ci, r0, rn)
                    o = ci * 512
                    cmm(acc[:mn], hre[:rn, m0:m0 + mn], him[:rn, m0:m0 + mn],
                        x1_m[:rn, o:o + 512], x1_m[:rn, o:o + 256],
                        x1_n[:rn, ci * 256:(ci + 1) * 256], ci, ci == 7)
                o = mi * 512
                nc.vector.tensor_copy(x2_m[:mn, o:o + 512], acc[:mn])
                nc.vector.tensor_scalar_mul(x2_n[:mn, mi * 256:(mi + 1) * 256],
                                            acc[:mn, 256:512], -1.0)

            # ---- step3: E = Winv @ X2 ----
            ps_t = sp.tile([128, 8 * 256], F32, tag="pst")
            for mi, (m0, mn) in enumerate(KC_M):
                acc = pp.tile([128, 512], F32, tag="accC")
                for ci, (r0, rn) in enumerate(KC_M):
                    lre = wp.tile([128, 128], F32R, tag="lre")
                    lim = wp.tile([128, 128], F32R, tag="lim")
                    nc.sync.dma_start(lre[:rn, :mn], ins["winv_re"][r0:r0 + rn, m0:m0 + mn])
                    nc.sync.dma_start(lim[:rn, :mn], ins["winv_im"][r0:r0 + rn, m0:m0 + mn])
                    o = ci * 512
                    cmm(acc[:mn], lre[:rn, :mn], lim[:rn, :mn],
                        x2_m[:rn, o:o + 512], x2_m[:rn, o:o + 256],
                        x2_n[:rn, ci * 256:(ci + 1) * 256], ci, ci == 7)
                # psf rows = |E|^2
                e_sb = tg.tile([128, 512], F32, tag="esb")
                nc.vector.tensor_copy(e_sb[:mn], acc[:mn])
                sq = tg.tile([128, 256], F32, tag="sq")
                nc.vector.tensor_tensor(sq[:mn], e_sb[:mn, 0:256], e_sb[:mn, 0:256], ALU.mult)
                sq2 = tg.tile([128, 256], F32, tag="sq2")
                nc.vector.tensor_tensor(sq2[:mn], e_sb[:mn, 256:512], e_sb[:mn, 256:512], ALU.mult)
                nc.vector.tensor_tensor(ps_t[:mn, mi * 256:(mi + 1) * 256],
                                        sq[:mn], sq2[:mn], ALU.add)
            for mi, (m0, mn) in enumerate(KC_M):
                nc.sync.dma_start(out_psf[m0:m0 + mn, :], ps_t[:mn, mi * 256:(mi + 1) * 256])

    nc.compile()
    _NCA["nc"] = nc
    return nc


def _launchA_inputs(md):
    C = _consts()
    k_arr = C["k_arr"]
    m2 = np.float32(md * md)
    r_supp = np.sqrt((C["R2supp"] + m2).astype(np.float32)).astype(np.float32)
    in_maps = []
    for core in range(NCORE):
        ch = min(core // 2, 2)
        half = core % 2
        c0 = half * 256
        n = min(256, NS - c0)
        kc = k_arr[ch]
        t1 = np.float32(kc * md)
        qch = C["qs"][ch]
        sc = np.zeros((128, 16), np.float32)
        sc[:, 0] = kc
        sc[:, 1] = t1
        rs = np.zeros((NS, 256), np.float32)
        rs[:, :n] = r_supp[:, c0:c0 + n]
        msk = np.zeros((NS, 256), np.float32)
        msk[:, :n] = C["A_supp"][:, c0:c0 + n]
        in_maps.append({
            "qs": np.ascontiguousarray(qch), "rs": rs, "msk": msk, "sc": sc,
        })
    return in_maps


def _assemble_psf(results):
    C = _consts()
    perm_s, perm_si = C["perm_s"], C["perm_si"]
    psf = np.zeros((3, MX, MX), np.float64)
    for core in range(6):
        ch = core // 2
        half = core % 2
        c0 = half * 256
        n = min(256, NS - c0)
        blk = results[core]["psf"][:MX, :]
        psf[ch][:, S0 + c0:S0 + c0 + n] = blk[:, :n]
    colmap = perm_s[perm_si]
    return psf[:, perm_si][:, :, colmap]


# ---------------------------------------------------------------------------
# Launch B
# ---------------------------------------------------------------------------
_NCB = {}


def _build_B():
    if "nc" in _NCB:
        return _NCB["nc"]
    nc = bacc.Bacc("TRN2", target_bir_lowering=False, debug=False)
    C = _consts()
    ins = {}
    ins["wgt_re"] = nc.inline_tensor(C["WgT_re"], "wgt_re").ap().bitcast(F32R)
    ins["wgt_im"] = nc.inline_tensor(C["WgT_im"], "wgt_im").ap().bitcast(F32R)
    for u in range(3):
        ins[f"wrow{u}"] = nc.dram_tensor(f"wrow{u}", [MX, 256], F32R, kind="ExternalInput").ap()
        ins[f"wl_re{u}"] = nc.dram_tensor(f"wl_re{u}", [128, 512], F32R, kind="ExternalInput").ap()
        ins[f"wl_imN{u}"] = nc.dram_tensor(f"wl_imN{u}", [128, 512], F32R, kind="ExternalInput").ap()
    for s in range(2):
        ins[f"img{s}"] = nc.dram_tensor(f"img{s}", [MX, MX], F32R, kind="ExternalInput").ap()
        ins[f"psf{s}"] = nc.dram_tensor(f"psf{s}", [MX, MX], F32R, kind="ExternalInput").ap()
    ins["wrt_re"] = nc.inline_tensor(C["WRT_re"], "wrt_re").ap().bitcast(F32R)
    ins["wrt_im"] = nc.inline_tensor(C["WRT_im"], "wrt_im").ap().bitcast(F32R)
    ins["wrt_imN"] = nc.inline_tensor(C["WRT_imN"], "wrt_imN").ap().bitcast(F32R)
    ins["ident"] = nc.inline_tensor(C["ident"], "ident").ap()
    pout = nc.dram_tensor("pout", [3 * 512, RES], F32, kind="ExternalOutput").ap()

    with tile.TileContext(nc) as tc:
        with (
            tc.tile_pool(name="cst", bufs=1) as cp,
            tc.tile_pool(name="stt", bufs=2) as st,
            tc.tile_pool(name="st1", bufs=1) as sq_,
            tc.tile_pool(name="wts", bufs=3) as wp,
            tc.tile_pool(name="tmp", bufs=2) as tp_,
            tc.tile_pool(name="psB", bufs=1, space="PSUM") as pp,
        ):
            ident = cp.tile([128, 128], F32, tag="ident")
            nc.sync.dma_start(ident[:], ins["ident"][:])

            # ---- inner passes: units 0,1 share slot 0 -> fused N=512 pass;
            #      unit 2 (slot 1) runs alone at N=256. tmpT_{u}{side} in a
            #      bufs=1 pool (written once, read through the unit loop). ----
            tmps_all = {}
            wrowF = sq_.tile([128, 8 * 512], F32R, tag="wrowF")
            for mi, (m0, mn) in enumerate(KC_M):
                nc.sync.dma_start(wrowF[:mn, mi * 512:mi * 512 + 256],
                                  ins["wrow0"][m0:m0 + mn, :])
                nc.sync.dma_start(wrowF[:mn, mi * 512 + 256:mi * 512 + 512],
                                  ins["wrow1"][m0:m0 + mn, :])
            for side in ("i", "p"):
                src = ins["img0"] if side == "i" else ins["psf0"]
                tt0 = sq_.tile([128, 8 * 256], F32R, tag=f"tmpT0{side}")
                tt1 = sq_.tile([128, 8 * 256], F32R, tag=f"tmpT1{side}")
                for mi, (m0, mn) in enumerate(KC_M):
                    acc = pp.tile([128, 512], F32, tag="accRei")
                    for ci, (r0, rn) in enumerate(KC_M):
                        dat = wp.tile([128, 128], F32R, tag="dat")
                        nc.sync.dma_start(dat[:rn, :mn], src[r0:r0 + rn, m0:m0 + mn])
                        nc.tensor.matmul(acc[:mn], dat[:rn, :mn],
                                         wrowF[:rn, ci * 512:(ci + 1) * 512],
                                         start=(ci == 0), stop=(ci == 7))
                    nc.vector.tensor_copy(tt0[:mn, mi * 256:(mi + 1) * 256], acc[:mn, 0:256])
                    nc.vector.tensor_copy(tt1[:mn, mi * 256:(mi + 1) * 256], acc[:mn, 256:512])
                tmps_all[(0, side)] = tt0
                tmps_all[(1, side)] = tt1
            wrow2 = sq_.tile([128, 8 * 256], F32R, tag="wrow")
            for mi, (m0, mn) in enumerate(KC_M):
                nc.sync.dma_start(wrow2[:mn, mi * 256:mi * 256 + 256],
                                  ins["wrow2"][m0:m0 + mn, :])
            for side in ("i", "p"):
                src = ins["img1"] if side == "i" else ins["psf1"]
                tt2 = sq_.tile([128, 8 * 256], F32R, tag=f"tmpT2{side}")
                for mi, (m0, mn) in enumerate(KC_M):
                    acc = pp.tile([128, 256], F32, tag="accRei")
                    for ci, (r0, rn) in enumerate(KC_M):
                        dat = wp.tile([128, 128], F32R, tag="dat")
                        nc.sync.dma_start(dat[:rn, :mn], src[r0:r0 + rn, m0:m0 + mn])
                        nc.tensor.matmul(acc[:mn], dat[:rn, :mn],
                                         wrow2[:rn, ci * 256:(ci + 1) * 256],
                                         start=(ci == 0), stop=(ci == 7))
                    nc.vector.tensor_copy(tt2[:mn, mi * 256:(mi + 1) * 256], acc[:mn])
                tmps_all[(2, side)] = tt2

            for u in range(3):
                tmps = {"i": tmps_all[(u, "i")], "p": tmps_all[(u, "p")]}
                # ---- step2 + product: D rows block [128 x 1536] ----
                dre = st.tile([128, P], F32, tag="dre")
                dim = st.tile([128, P], F32, tag="dim")
                for nt in range(3):
                    n0 = nt * 512
                    accs = {}
                    for side in ("i", "p"):
                        acc_re = pp.tile([128, 512], F32, tag=f"accRe{side}")
                        acc_im = pp.tile([128, 512], F32, tag=f"accIm{side}")
                        accs[side] = (acc_re, acc_im)
                    for ci, (r0, rn) in enumerate(KC_M):
                        wgr = wp.tile([128, 512], F32R, tag="wgr")
                        wgi = wp.tile([128, 512], F32R, tag="wgi")
                        nc.sync.dma_start(wgr[:rn], ins["wgt_re"][r0:r0 + rn, n0:n0 + 512])
                        nc.sync.dma_start(wgi[:rn], ins["wgt_im"][r0:r0 + rn, n0:n0 + 512])
                        for side in ("i", "p"):
                            tt = tmps[side]
                            a_re, a_im = accs[side]
                            tre = tt[:rn, ci * 256:ci * 256 + 128]
                            tim = tt[:rn, ci * 256 + 128:ci * 256 + 256]
                            timn = tp_.tile([128, 128], F32R, tag="timn")
                            nc.vector.tensor_scalar_mul(timn[:rn], tim.bitcast(F32), -1.0)
                            nc.tensor.matmul(a_re[:], tre, wgr[:rn], start=(ci == 0), stop=False)
                            nc.tensor.matmul(a_re[:], timn[:rn], wgi[:rn], start=False,
                                             stop=(ci == 7))
                            nc.tensor.matmul(a_im[:], tre, wgi[:rn], start=(ci == 0), stop=False)
                            nc.tensor.matmul(a_im[:], tim, wgr[:rn], start=False, stop=(ci == 7))
                    fir = tp_.tile([128, 512], F32, tag="fir")
                    fii = tp_.tile([128, 512], F32, tag="fii")
                    nc.vector.tensor_copy(fir[:], accs["i"][0][:])
                    nc.vector.tensor_copy(fii[:], accs["i"][1][:])
                    fpr, fpi = accs["p"]
                    t1_ = tp_.tile([128, 512], F32, tag="pr1")
                    t2_ = tp_.tile([128, 512], F32, tag="pr2")
                    nc.vector.tensor_tensor(t1_[:], fir[:], fpr[:], ALU.mult)
                    nc.vector.tensor_tensor(t2_[:], fii[:], fpi[:], ALU.mult)
                    nc.vector.tensor_tensor(dre[:, n0:n0 + 512], t1_[:], t2_[:], ALU.subtract)
                    nc.vector.tensor_tensor(t1_[:], fir[:], fpi[:], ALU.mult)
                    nc.vector.tensor_tensor(t2_[:], fii[:], fpr[:], ALU.mult)
                    nc.vector.tensor_tensor(dim[:, n0:n0 + 512], t1_[:], t2_[:], ALU.add)
                # ---- transpose D -> stationary ----
                dT_re = st.tile([128, 12 * 128], F32R, tag="dTre")
                dT_im = st.tile([128, 12 * 128], F32R, tag="dTim")
                dT_imN = st.tile([128, 12 * 128], F32R, tag="dTimN")
                for ci in range(12):
                    sl = slice(ci * 128, (ci + 1) * 128)
                    for plane, dst in ((dre, dT_re), (dim, dT_im)):
                        ptr = pp.tile([128, 128], F32, tag="accO")
                        nc.tensor.transpose(ptr[:], plane[:, sl], ident[:])
                        nc.vector.tensor_copy(dst[:, sl], ptr[:])
                    nc.vector.tensor_scalar_mul(dT_imN[:, sl], dT_im[:, sl].bitcast(F32), -1.0)
                # ---- s1: Y = D^T @ WR^T ----
                y_re_p = pp.tile([128, 512], F32, tag="yre")
                y_im_p = pp.tile([128, 512], F32, tag="yim")
                for ci, (r0, rn) in enumerate(KC_P):
                    wrr = wp.tile([128, 512], F32R, tag="wrr")
                    wri = wp.tile([128, 512], F32R, tag="wri")
                    wrn = wp.tile([128, 512], F32R, tag="wrn")
                    nc.sync.dma_start(wrr[:], ins["wrt_re"][r0:r0 + rn, :])
                    nc.sync.dma_start(wri[:], ins["wrt_im"][r0:r0 + rn, :])
                    nc.sync.dma_start(wrn[:], ins["wrt_imN"][r0:r0 + rn, :])
                    sl = slice(ci * 128, (ci + 1) * 128)
                    nc.tensor.matmul(y_re_p[:], dT_re[:, sl], wrr[:], start=(ci == 0), stop=False)
                    nc.tensor.matmul(y_re_p[:], dT_imN[:, sl], wri[:], start=False, stop=(ci == 11))
                    nc.tensor.matmul(y_im_p[:], dT_re[:, sl], wri[:], start=(ci == 0), stop=False)
                    nc.tensor.matmul(y_im_p[:], dT_im[:, sl], wrr[:], start=False, stop=(ci == 11))
                y_re = st.tile([128, 512], F32R, tag="yreS")
                y_im = st.tile([128, 512], F32R, tag="yimS")
                nc.vector.tensor_copy(y_re[:], y_re_p[:])
                nc.vector.tensor_copy(y_im[:], y_im_p[:])
                # ---- partial: pout_u = WLre@Yre + WLimN@Yim ----
                wlre = st.tile([128, 512], F32R, tag="wlre")
                wlim = st.tile([128, 512], F32R, tag="wlim")
                nc.sync.dma_start(wlre[:], ins[f"wl_re{u}"][:])
                nc.sync.dma_start(wlim[:], ins[f"wl_imN{u}"][:])
                for mt in range(4):
                    mrows = 128 if mt < 3 else 127
                    accO = pp.tile([128, 512], F32, tag="accO")
                    nc.tensor.matmul(accO[:mrows], wlre[:, mt * 128:mt * 128 + mrows], y_re[:],
                                     start=True, stop=False)
                    nc.tensor.matmul(accO[:mrows], wlim[:, mt * 128:mt * 128 + mrows], y_im[:],
                                     start=False, stop=True)
                    oo = tp_.tile([128, 512], F32, tag="oo")
                    nc.vector.tensor_copy(oo[:mrows], accO[:mrows])
                    nc.sync.dma_start(pout[u * 512 + mt * 128:u * 512 + mt * 128 + mrows, :],
                                      oo[:mrows, :RES])

    nc.compile()
    _NCB["nc"] = nc
    return nc


def _unit_order():
    units = [(g // 8, g % 8) for g in range(24)]
    per_core = []
    for core in range(NCORE):
        us = units[3 * core:3 * core + 3]
        if us[0][0] != us[1][0]:
            us = [us[1], us[2], us[0]]
        assert us[0][0] == us[1][0]
        per_core.append(us)
    return per_core


_B_STATIC = {}


def _launchB_inputs(psf_hi, img_hi):
    C = _consts()
    Wg, WLz = C["Wg"], C["WLz"]
    pcu = _unit_order()
    if not _B_STATIC:
        mats = []
        for core in range(NCORE):
            us = pcu[core]
            m = {}
            for u, (c, b) in enumerate(us):
                rows = slice(128 * b, 128 * (b + 1))
                wr = Wg[rows, :].T
                wrow = np.empty((MX, 256), np.float32)
                wrow[:, :128] = _r32c(np.real(wr))
                wrow[:, 128:] = _r32c(np.imag(wr))
                m[f"wrow{u}"] = wrow
                wl = WLz[:, rows]
                wlre = np.zeros((128, 512), np.float32)
                wlim = np.zeros((128, 512), np.float32)
                wlre[:, :RES] = _r32c(np.real(wl).T)
                wlim[:, :RES] = _r32c(-np.imag(wl).T)
                m[f"wl_re{u}"] = wlre
                m[f"wl_imN{u}"] = wlim
            mats.append(m)
        _B_STATIC["mats"] = mats
    in_maps = []
    for core in range(NCORE):
        us = pcu[core]
        slot_ch = [us[0][0], us[2][0]]
        m = dict(_B_STATIC["mats"][core])
        for s in range(2):
            m[f"img{s}"] = img_hi[slot_ch[s]]
            m[f"psf{s}"] = psf_hi[slot_ch[s]]
        in_maps.append(m)
    return in_maps, pcu


LAST_TIMES = {}


def kernel(image, depth):
    import time as _time
    image = np.asarray(image, np.float32)
    depth = np.asarray(depth, np.float32)
    try:
        import jax
        import jax.numpy as jnp
        cpu = jax.devices("cpu")[0]
        with jax.default_device(cpu):
            md = np.float32(jax.jit(jnp.mean, backend="cpu")(jax.device_put(depth, cpu)))
    except Exception:
        md = np.float32(np.sum(depth.ravel(), dtype=np.float32) / np.float32(depth.size))

    ncA = _build_A()
    _t0 = _time.time()
    resA = run_bass_kernel_spmd(ncA, _launchA_inputs(md), list(range(NCORE)))
    LAST_TIMES["A"] = _time.time() - _t0
    psf = _assemble_psf(resA.results)
    Sp = np.float32(np.float32(psf.sum()) + np.float32(1e-7))
    kflip = np.ascontiguousarray(psf[:, ::-1, ::-1]).astype(np.float32)
    psf_hi = np.stack([_r32c(kflip[c]) for c in range(3)])
    img_hi = np.stack([_r32c(image[c]) for c in range(3)])

    ncB = _build_B()
    in_maps, pcu = _launchB_inputs(psf_hi, img_hi)
    _t0 = _time.time()
    resB = run_bass_kernel_spmd(ncB, in_maps, list(range(NCORE)))
    LAST_TIMES["B"] = _time.time() - _t0
    out = np.zeros((3, RES, RES), np.float64)
    for core in range(NCORE):
        po = resB.results[core]["pout"]
        for u, (c, b) in enumerate(pcu[core]):
            out[c] += po[u * 512:u * 512 + RES, :]
    out = out / np.float64(Sp)
    return np.clip(out, 0.0, 1.0).astype(np.float32)



# revision 10
# speedup vs baseline: 16.4346x; 16.4346x over previous
"""Trainium2 Bass kernel for nn_MjCambrianOptics (depth-invariant PSF + FFT blur).

Single fused SPMD launch on 8 cores (tunnel-payload optimized).

Every core computes the FULL 3-channel PSF on device (replicated: PE time is
cheap, tunnel bytes are not) via the launch-A algebra E = Winv @ Hs @ W1s @ u2
with bit-exact phases fl(k*r), fl(t1*q) (host IEEE sqrt r ships as input;
Cody-Waite reduction + ACT Sin on device). PSF matmuls run in bf16 (incoherent
rounding noise ~2e-3 ≪ 2e-2 tolerance).

The conv side absorbs the reference's psf flip + fftshift row-perm into
statically reordered twiddles: Fpsf[r,c] = Σ P[i,jj] w^{r(1022-perm_s[i])}
w^{c(766-jj)} (only the 511 nonzero psf cols enter). The image is row-sharded
1/8 per core in bf16; Fimg factors as TwL^T @ (imgT @ WgT) with the global row
offset phase w^{r*128*core} deferred to a per-partition complex scale on Y.
Everything downstream is linear in the image, so per-core fp16 partial outputs
sum on the host. Hermitian row truncation (WLz doubling) keeps r-blocks 0..6.

Per-core payload: img slice bf16 0.79MB + r_supp f32 1.05MB + scal; out
fp16 1.57MB. ~28MB total vs ~250MB for the two-launch baseline.
"""
import numpy as np
import ml_dtypes

import concourse.bacc as bacc
import concourse.mybir as mybir
import concourse.tile as tile
from concourse.bass_utils import run_bass_kernel_spmd

F32 = mybir.dt.float32
F32R = mybir.dt.float32r
BF16 = mybir.dt.bfloat16
F16 = mybir.dt.float16
AF = mybir.ActivationFunctionType
ALU = mybir.AluOpType
AX = mybir.AxisListType

MX = 1023
RES = 511
S0 = 256
NS = 511
SENSOR = 0.01
APERTURE = 0.5
WAVELENGTHS = np.array([610e-9, 530e-9, 470e-9], dtype=np.float32)
P = 1536
NRB = 7                 # freq row blocks 0..6 (896 rows >= 769 Hermitian rows)
NCORE = 8

PI = np.float32(np.pi)
TWO_PI = np.float64(2.0) * np.pi
C_RND = float(np.float32(1.5 * 2.0 ** 23))

KC_S = [(0, 128), (128, 128), (256, 128), (384, 127)]        # 511 rows
KC_M = [(i * 128, 128) for i in range(7)] + [(896, 127)]     # 1023 rows


def _r32c(x):
    """Round ndarray to f32r (12-bit significand), RNE — matches tensor_copy."""
    f = np.ascontiguousarray(x, np.float32)
    b = f.view(np.uint32).astype(np.uint64)
    low = b & 0xFFF
    b2 = b & ~np.uint64(0xFFF)
    up = (low > 0x800) | ((low == 0x800) & (((b2 >> 12) & 1) == 1))
    b2 = b2 + np.where(up, np.uint64(0x1000), np.uint64(0))
    return b2.astype(np.uint32).view(np.float32).reshape(f.shape)


def _bf16(x):
    return np.ascontiguousarray(x, np.float32).astype(ml_dtypes.bfloat16)


def _splitb(x64, keep):
    f = np.float32(x64)
    mask = np.uint32(0xFFFFFFFF ^ ((1 << (24 - keep)) - 1))
    bits = np.uint32(int(f.view(np.uint32)) & int(mask))
    return bits.view(np.float32)


P1 = _splitb(TWO_PI, 11)
P2 = _splitb(TWO_PI - np.float64(P1), 10)
P3 = np.float32(TWO_PI - np.float64(P1) - np.float64(P2))
P1H = np.float32(np.float64(P1) * 256.0)
P2H = np.float32(np.float64(P2) * 256.0)

_CONSTS = {}


def _consts():
    if _CONSTS:
        return _CONSTS
    dx = SENSOR / MX
    Lx = dx * MX
    x1 = np.linspace(-Lx / 2, Lx / 2, MX, dtype=np.float32)
    X1, Y1 = np.meshgrid(x1, x1, indexing="ij")
    fx = np.linspace(-1.0 / (2 * dx), 1.0 / (2 * dx), MX, dtype=np.float32)
    FX, FY = np.meshgrid(fx, fx, indexing="ij")
    ar = (Lx / 2.0) * APERTURE
    A = (np.sqrt(X1 ** 2 + Y1 ** 2) / np.float32(ar + 1e-7) <= 1.0).astype(np.float32)
    lam = WAVELENGTHS
    k_arr = (np.float32(2.0) * np.float32(np.pi) / lam).astype(np.float32)
    q = np.empty((3, MX, MX), np.float32)
    for c in range(3):
        a_ = (lam[c] * FX).astype(np.float32)
        b_ = (lam[c] * FY).astype(np.float32)
        s_ = ((np.float32(1.0) - (a_ * a_).astype(np.float32)).astype(np.float32)
              - (b_ * b_).astype(np.float32)).astype(np.float32)
        q[c] = np.sqrt(s_).astype(np.float32)
    R2 = ((X1 * X1).astype(np.float32) + (Y1 * Y1).astype(np.float32)).astype(np.float32)

    jk = np.arange(MX)
    W = np.exp(-2j * np.pi * np.outer(jk, jk) / MX)
    Winv = np.conj(W) / MX
    perm_s = (jk - MX // 2) % MX
    perm_si = (jk + MX // 2) % MX
    WS_s = W[:, perm_si][:, S0:S0 + NS]        # [1023 x 511]
    qs_sh = q[:, perm_s][:, :, perm_s]         # Hs = exp(i t1 qs_sh)

    jP = np.arange(P)
    Wp = np.exp(-2j * np.pi * np.outer(jP, jP) / P)
    Winvp = np.conj(Wp) / P
    selr = 767 + np.arange(RES)
    WLm = Winvp[selr, :769].copy()
    WLm[:, 1:768] *= 2.0
    WLz = np.zeros((RES, NRB * 128), np.complex128)
    WLz[:, :769] = WLm
    WR = Winvp[selr, :]                        # [511 x 1536]

    kvec = 1022 - perm_s                       # psf row exponent map
    mvec = 766 - np.arange(NS)                 # psf col exponent map

    C = {}
    C["k_arr"] = k_arr
    C["R2supp"] = R2[S0:S0 + NS, S0:S0 + NS]
    amask = np.zeros((512, 512), np.float32)
    amask[:NS, :NS] = A[S0:S0 + NS, S0:S0 + NS]
    C["amask"] = amask
    for c in range(3):
        C[f"qs{c}"] = np.ascontiguousarray(qs_sh[c])

    w1p = np.zeros((NS, 2048), np.float32)
    w1p[:, :MX] = np.real(WS_s).T
    w1p[:, 1024:1024 + MX] = np.imag(WS_s).T
    C["w1s_pack"] = _bf16(w1p)                 # [511 x 2048] bf16 lhsT step1
    wvp = np.zeros((MX, 2048), np.float32)
    wvp[:, :MX] = np.real(Winv)                # symmetric
    wvp[:, 1024:1024 + MX] = np.imag(Winv)
    C["winv_pack"] = _bf16(wvp)

    wgpt = np.zeros((MX, 1792), np.float64)    # [i x (re 896 | im 896)]
    WgP = Wp[:896, :][:, kvec]                 # [896 r x 1023 i]
    wgpt[:, :896] = np.real(WgP).T
    wgpt[:, 896:1792] = np.imag(WgP).T
    C["wgpt_pack"] = _r32c(wgpt)

    wgq = np.zeros((NS, 4608), np.float64)     # [jj x (re|im|imN 1536)]
    WgQ = Wp[mvec, :]                          # [511 jj x 1536 c]
    wgq[:, :1536] = np.real(WgQ)
    wgq[:, 1536:3072] = np.imag(WgQ)
    wgq[:, 3072:4608] = -np.imag(WgQ)
    C["wgq_pack"] = _r32c(wgq)

    wgt = np.zeros((MX, 3072), np.float64)     # [m x (re|im 1536)] w^{cm}
    WgT = Wp[:, :MX].T
    wgt[:, :1536] = np.real(WgT)
    wgt[:, 1536:3072] = np.imag(WgT)
    C["wgt_pack"] = _r32c(wgt)

    twl = np.zeros((128, 2688), np.float64)    # [k' x (re|im|imN 896)] w^{r k'}
    TwL = Wp[:128, :896]
    twl[:, :896] = np.real(TwL)
    twl[:, 896:1792] = np.imag(TwL)
    twl[:, 1792:2688] = -np.imag(TwL)
    C["twl_pack"] = _r32c(twl)

    wrt = np.zeros((P, 1536), np.float64)      # [c x (re|im|imN 512)] WR^T
    wrt[:, :RES] = np.real(WR).T
    wrt[:, 512:512 + RES] = np.imag(WR).T
    wrt[:, 1024:1024 + RES] = -np.imag(WR).T
    C["wrt_pack"] = _r32c(wrt)

    wlz = np.zeros((NRB * 128, 1024), np.float64)  # [r x (re 512 | imN 512)]
    wlz[:, :RES] = np.real(WLz).T * (2.0 ** -26)   # fp16-range scale, exact
    wlz[:, 512:512 + RES] = -np.imag(WLz).T * (2.0 ** -26)
    C["wlzt_pack"] = _r32c(wlz)

    C["ident"] = np.eye(128, dtype=np.float32)
    C["Wp"] = Wp
    _CONSTS.update(C)
    return _CONSTS


# ---------------------------------------------------------------------------
_NC = {}


def _build():
    if "nc" in _NC:
        return _NC["nc"]
    nc = bacc.Bacc("TRN2", target_bir_lowering=False, debug=False)
    C = _consts()
    ins = {}
    for nm in ["wgpt_pack", "wgq_pack", "wgt_pack", "twl_pack", "wrt_pack",
               "wlzt_pack"]:
        ins[nm] = nc.inline_tensor(C[nm], nm).ap().bitcast(F32R)
    for nm in ["w1s_pack", "winv_pack"]:
        ins[nm] = nc.inline_tensor(C[nm], nm).ap()
    for c in range(3):
        ins[f"qs{c}"] = nc.inline_tensor(C[f"qs{c}"], f"qs{c}").ap()
    ins["amask"] = nc.inline_tensor(C["amask"], "amask").ap()
    ins["ident"] = nc.inline_tensor(C["ident"], "ident").ap()
    ins["rs"] = nc.dram_tensor("rs", [512, 512], F32, kind="ExternalInput").ap()
    ins["img"] = nc.dram_tensor("img", [384, 1024], BF16, kind="ExternalInput").ap()
    ins["sc"] = nc.dram_tensor("sc", [128, 32], F32, kind="ExternalInput").ap()
    pout = nc.dram_tensor("pout", [1536, 512], F16, kind="ExternalOutput").ap()
    psums = nc.dram_tensor("psums", [128, 4], F32, kind="ExternalOutput").ap()

    with tile.TileContext(nc) as tc:
        with (
            tc.tile_pool(name="cst", bufs=1) as cp,
            tc.tile_pool(name="acc", bufs=1) as ao,
        ):
            scal = cp.tile([128, 32], F32, tag="scal")
            nc.sync.dma_start(scal[:], ins["sc"][:])
            ident = cp.tile([128, 128], F32, tag="ident")
            nc.sync.dma_start(ident[:], ins["ident"][:])
            lrs = cp.tile([128, 4 * 512], F32, tag="lrs")
            lmsk = cp.tile([128, 4 * 512], F32, tag="lmsk")
            for ci, (r0, rn) in enumerate(KC_S):
                nc.sync.dma_start(lrs[:rn, ci * 512:(ci + 1) * 512],
                                  ins["rs"][r0:r0 + rn, :])
                nc.sync.dma_start(lmsk[:rn, ci * 512:(ci + 1) * 512],
                                  ins["amask"][r0:r0 + rn, :])
            twl = cp.tile([128, 2688], F32R, tag="twl")
            nc.sync.dma_start(twl[:], ins["twl_pack"][:])
            oacc = ao.tile([128, 3 * 2048], F32, tag="oacc")
            nc.vector.memset(oacc[:], 0.0)

            def trig_pair(dst_cos, dst_sin, base_ap, t_col, rows, w,
                          tg, mask_ap=None):
                th = tg.tile([128, MX], F32, tag="th")
                nc.vector.tensor_scalar_mul(th[:rows, :w], base_ap,
                                            scal[:rows, t_col:t_col + 1])
                f = tg.tile([128, MX], F32, tag="f")
                nc.vector.tensor_scalar(f[:rows, :w], th[:rows, :w],
                                        float(np.float32(1.0 / TWO_PI)), C_RND,
                                        ALU.mult, ALU.add)
                nc.vector.tensor_scalar_sub(f[:rows, :w], f[:rows, :w], C_RND)
                g = tg.tile([128, MX], F32, tag="g")
                nc.vector.tensor_scalar(g[:rows, :w], f[:rows, :w],
                                        float(np.float32(1.0 / 256.0)), C_RND,
                                        ALU.mult, ALU.add)
                nc.vector.tensor_scalar_sub(g[:rows, :w], g[:rows, :w], C_RND)
                nl = tg.tile([128, MX], F32, tag="nl")
                nc.vector.cody_waite_cascade(nl[:rows, :w], f[:rows, :w],
                                             g[:rows, :w], 256.0, 0.0, 0.0)
                y = tg.tile([128, MX], F32, tag="y")
                nc.vector.cody_waite_cascade(y[:rows, :w], th[:rows, :w],
                                             g[:rows, :w], float(P1H), float(P2H), 0.0)
                nc.vector.cody_waite_cascade(y[:rows, :w], y[:rows, :w],
                                             nl[:rows, :w], float(P1), float(P2), 0.0)
                nc.vector.cody_waite_cascade(y[:rows, :w], y[:rows, :w],
                                             f[:rows, :w], float(P3), 0.0, 0.0)
                yw = tg.tile([128, MX], F32, tag="yw")
                nc.vector.add_range_wrap(yw[:rows, :w], y[:rows, :w], 0.0,
                                         float(PI), float(np.float32(2 * np.pi)))
                yc = tg.tile([128, MX], F32, tag="yc")
                nc.vector.add_range_wrap(yc[:rows, :w], y[:rows, :w],
                                         float(np.float32(PI / 2)), float(PI),
                                         float(np.float32(2 * np.pi)))
                if mask_ap is None:
                    nc.scalar.activation(dst_sin, yw[:rows, :w], AF.Sin)
                    nc.scalar.activation(dst_cos, yc[:rows, :w], AF.Sin)
                else:
                    sn = tg.tile([128, 256], F32, tag="sn")
                    cn = tg.tile([128, 256], F32, tag="cn")
                    nc.scalar.activation(sn[:rows, :w], yw[:rows, :w], AF.Sin)
                    nc.scalar.activation(cn[:rows, :w], yc[:rows, :w], AF.Sin)
                    nc.vector.tensor_tensor(dst_sin, sn[:rows, :w], mask_ap, ALU.mult)
                    nc.vector.tensor_tensor(dst_cos, cn[:rows, :w], mask_ap, ALU.mult)

            for ch in range(3):
                with tc.tile_pool(name="chan", bufs=1) as chp:
                    ps_t = chp.tile([128, 8 * 512], F32R, tag="ps_t")
                    nc.vector.memset(ps_t[:].bitcast(F32), 0.0)

                    # ======== PSF phase: E = Winv @ Hs @ W1s @ u2 ========
                    with (
                        tc.tile_pool(name="psfp", bufs=1) as pf,
                        tc.tile_pool(name="trg", bufs=1) as tg,
                        tc.tile_pool(name="wbf", bufs=2) as wb,
                        tc.tile_pool(name="psA", bufs=2, space="PSUM") as pp,
                    ):
                        # Hs trig (resident bf16, 8 chunks)
                        hs = {}
                        for ci, (r0, rn) in enumerate(KC_M):
                            qsl = tg.tile([128, MX], F32, tag="qsl")
                            nc.sync.dma_start(qsl[:rn], ins[f"qs{ch}"][r0:r0 + rn, :])
                            hre = pf.tile([128, MX], BF16, tag=f"hre{ci}")
                            him = pf.tile([128, MX], BF16, tag=f"him{ci}")
                            trig_pair(hre[:rn], him[:rn], qsl[:rn], 3 + ch,
                                      rn, MX, tg)
                            hs[ci] = (hre, him)

                        # u2 trig: per chunk [h0re 256|h0im 256|h1re|h1im]
                        u2_m = pf.tile([128, 4 * 1024], BF16, tag="u2m")
                        u2_n = pf.tile([128, 4 * 512], BF16, tag="u2n")
                        nc.vector.memset(u2_m[:], 0.0)
                        for ci, (r0, rn) in enumerate(KC_S):
                            for h in range(2):
                                o = ci * 1024 + h * 512
                                base = lrs[:rn, ci * 512 + h * 256:
                                           ci * 512 + h * 256 + 256]
                                mk = lmsk[:rn, ci * 512 + h * 256:
                                          ci * 512 + h * 256 + 256]
                                trig_pair(u2_m[:rn, o:o + 256],
                                          u2_m[:rn, o + 256:o + 512],
                                          base, ch, rn, 256, tg, mask_ap=mk)
                            nc.vector.tensor_scalar_mul(
                                u2_n[:rn, ci * 512:ci * 512 + 256],
                                u2_m[:rn, ci * 1024 + 256:ci * 1024 + 512], -1.0)
                            nc.vector.tensor_scalar_mul(
                                u2_n[:rn, ci * 512 + 256:ci * 512 + 512],
                                u2_m[:rn, ci * 1024 + 768:ci * 1024 + 1024], -1.0)

                        def cmm(acc, lre, lim, m_full, m_re, m_im_neg, first, last):
                            nc.tensor.matmul(acc[:, 0:512], lre, m_full,
                                             start=first, stop=False)
                            nc.tensor.matmul(acc[:, 0:256], lim, m_im_neg,
                                             start=False, stop=False)
                            nc.tensor.matmul(acc[:, 256:512], lim, m_re,
                                             start=False, stop=last)

                        for h in range(2):
                            # ---- step1: X1 = W1s @ u2(half) ----
                            x1_m = pf.tile([128, 8 * 512], BF16, tag="x1m")
                            x1_n = pf.tile([128, 8 * 256], BF16, tag="x1n")
                            for mi, (m0, mn) in enumerate(KC_M):
                                acc = pp.tile([128, 512], F32, tag="accA")
                                for ci, (r0, rn) in enumerate(KC_S):
                                    lw = wb.tile([128, 2048], BF16, tag="wbf")
                                    nc.sync.dma_start(
                                        lw[:rn, :mn],
                                        ins["w1s_pack"][r0:r0 + rn, m0:m0 + mn])
                                    nc.sync.dma_start(
                                        lw[:rn, 1024:1024 + mn],
                                        ins["w1s_pack"][r0:r0 + rn,
                                                        1024 + m0:1024 + m0 + mn])
                                    o = ci * 1024 + h * 512
                                    cmm(acc[:mn], lw[:rn, :mn],
                                        lw[:rn, 1024:1024 + mn],
                                        u2_m[:rn, o:o + 512],
                                        u2_m[:rn, o:o + 256],
                                        u2_n[:rn, ci * 512 + h * 256:
                                             ci * 512 + h * 256 + 256],
                                        ci == 0, ci == 3)
                                o = mi * 512
                                nc.vector.tensor_copy(x1_m[:mn, o:o + 512], acc[:mn])
                                nc.vector.tensor_scalar_mul(
                                    x1_n[:mn, mi * 256:(mi + 1) * 256],
                                    acc[:mn, 256:512], -1.0)

                            # ---- step2: X2 = Hs @ X1 ----
                            x2_m = pf.tile([128, 8 * 512], BF16, tag="x2m")
                            x2_n = pf.tile([128, 8 * 256], BF16, tag="x2n")
                            for mi, (m0, mn) in enumerate(KC_M):
                                acc = pp.tile([128, 512], F32, tag="accB")
                                for ci, (r0, rn) in enumerate(KC_M):
                                    hre, him = hs[ci]
                                    o = ci * 512
                                    cmm(acc[:mn], hre[:rn, m0:m0 + mn],
                                        him[:rn, m0:m0 + mn],
                                        x1_m[:rn, o:o + 512],
                                        x1_m[:rn, o:o + 256],
                                        x1_n[:rn, ci * 256:(ci + 1) * 256],
                                        ci == 0, ci == 7)
                                o = mi * 512
                                nc.vector.tensor_copy(x2_m[:mn, o:o + 512], acc[:mn])
                                nc.vector.tensor_scalar_mul(
                                    x2_n[:mn, mi * 256:(mi + 1) * 256],
                                    acc[:mn, 256:512], -1.0)

                            # ---- step3: E = Winv @ X2; ps_t += |E|^2 ----
                            for mi, (m0, mn) in enumerate(KC_M):
                                acc = pp.tile([128, 512], F32, tag="accC")
                                for ci, (r0, rn) in enumerate(KC_M):
                                    lw = wb.tile([128, 2048], BF16, tag="wbf")
                                    nc.sync.dma_start(
                                        lw[:rn, :mn],
                                        ins["winv_pack"][r0:r0 + rn, m0:m0 + mn])
                                    nc.sync.dma_start(
                                        lw[:rn, 1024:1024 + mn],
                                        ins["winv_pack"][r0:r0 + rn,
                                                         1024 + m0:1024 + m0 + mn])
                                    o = ci * 512
                                    cmm(acc[:mn], lw[:rn, :mn],
                                        lw[:rn, 1024:1024 + mn],
                                        x2_m[:rn, o:o + 512],
                                        x2_m[:rn, o:o + 256],
                                        x2_n[:rn, ci * 256:(ci + 1) * 256],
                                        ci == 0, ci == 7)
                                e_sb = tg.tile([128, 512], F32, tag="esb")
                                nc.vector.tensor_copy(e_sb[:mn], acc[:mn])
                                sq = tg.tile([128, 256], F32, tag="sq")
                                nc.vector.tensor_tensor(sq[:mn], e_sb[:mn, 0:256],
                                                        e_sb[:mn, 0:256], ALU.mult)
                                sq2 = tg.tile([128, 256], F32, tag="sq2")
                                nc.vector.tensor_tensor(sq2[:mn], e_sb[:mn, 256:512],
                                                        e_sb[:mn, 256:512], ALU.mult)
                                nc.vector.tensor_tensor(
                                    ps_t[:mn, mi * 512 + h * 256:
                                         mi * 512 + h * 256 + 256],
                                    sq[:mn], sq2[:mn], ALU.add)

                    # psf channel sum -> psums[:, ch]
                    rsum = cp.tile([128, 4], F32, tag="rsum")
                    nc.vector.tensor_reduce(rsum[:, ch:ch + 1],
                                            ps_t[:].bitcast(F32), AX.X, ALU.add)
                    nc.sync.dma_start(psums[:, ch:ch + 1], rsum[:, ch:ch + 1])

                    # ======== CONV phase ========
                    with (
                        tc.tile_pool(name="convp", bufs=1) as cv,
                        tc.tile_pool(name="str", bufs=2) as stp,
                    ):
                        # ---- G = imgT @ WgT (row-DFT of this core's slice) ----
                        ibf = cv.tile([128, 1024], BF16, tag="ibf")
                        nc.sync.dma_start(ibf[:], ins["img"][ch * 128:(ch + 1) * 128, :])
                        if32 = cv.tile([128, 1024], F32, tag="if32")
                        nc.vector.tensor_copy(if32[:], ibf[:])
                        imgT = cv.tile([128, 1024], F32R, tag="imgT")
                        g_re = cv.tile([128, 1536], F32R, tag="g_re")
                        g_im = cv.tile([128, 1536], F32R, tag="g_im")
                        with tc.tile_pool(name="psG", bufs=1, space="PSUM") as pg:
                            for b in range(8):
                                ptr = pg.tile([128, 128], F32, tag="ptr")
                                nc.tensor.transpose(ptr[:], if32[:, b * 128:(b + 1) * 128],
                                                    ident[:])
                                nc.vector.tensor_copy(imgT[:, b * 128:(b + 1) * 128],
                                                      ptr[:])
                            gacc = {}
                            for seg in range(3):
                                gacc[seg] = (pg.tile([128, 512], F32, tag=f"gr{seg}", name=f"gr{seg}"),
                                             pg.tile([128, 512], F32, tag=f"gi{seg}", name=f"gi{seg}"))
                            for mi, (m0, mn) in enumerate(KC_M):
                                sw = stp.tile([128, 4608], F32R, tag="str")
                                nc.sync.dma_start(sw[:mn, :3072],
                                                  ins["wgt_pack"][m0:m0 + mn, :])
                                for seg in range(3):
                                    nc.tensor.matmul(
                                        gacc[seg][0][:], imgT[:mn, mi * 128:mi * 128 + 128],
                                        sw[:mn, seg * 512:seg * 512 + 512],
                                        start=(mi == 0), stop=(mi == 7))
                                    nc.tensor.matmul(
                                        gacc[seg][1][:], imgT[:mn, mi * 128:mi * 128 + 128],
                                        sw[:mn, 1536 + seg * 512:1536 + seg * 512 + 512],
                                        start=(mi == 0), stop=(mi == 7))
                            for seg in range(3):
                                nc.vector.tensor_copy(g_re[:, seg * 512:(seg + 1) * 512],
                                                      gacc[seg][0][:])
                                nc.vector.tensor_copy(g_im[:, seg * 512:(seg + 1) * 512],
                                                      gacc[seg][1][:])

                        # ---- tmpT_p[jj, r] = sum_i P[i,jj] WgP[r,i] ----
                        tpp = [cv.tile([128, 1792], F32R, tag=f"tpp{j}", name=f"tpp{j}")
                               for j in range(4)]
                        with tc.tile_pool(name="psT", bufs=2, space="PSUM") as pg:
                            for rs0, rw in ((0, 512), (512, 384)):
                                for jc, (j0, jn) in enumerate(KC_S):
                                    a_re = pg.tile([128, 512], F32, tag="tp_re")
                                    a_im = pg.tile([128, 512], F32, tag="tp_im")
                                    for ic, (i0, icn) in enumerate(KC_M):
                                        sw = stp.tile([128, 4608], F32R, tag="str")
                                        nc.sync.dma_start(
                                            sw[:icn, :rw],
                                            ins["wgpt_pack"][i0:i0 + icn, rs0:rs0 + rw])
                                        nc.sync.dma_start(
                                            sw[:icn, 1024:1024 + rw],
                                            ins["wgpt_pack"][i0:i0 + icn,
                                                             896 + rs0:896 + rs0 + rw])
                                        lhs = ps_t[:icn, ic * 512 + j0:ic * 512 + j0 + jn]
                                        nc.tensor.matmul(a_re[:jn, :rw], lhs,
                                                         sw[:icn, :rw],
                                                         start=(ic == 0), stop=(ic == 7))
                                        nc.tensor.matmul(a_im[:jn, :rw], lhs,
                                                         sw[:icn, 1024:1024 + rw],
                                                         start=(ic == 0), stop=(ic == 7))
                                    nc.vector.tensor_copy(
                                        tpp[jc][:jn, rs0:rs0 + rw], a_re[:jn, :rw])
                                    nc.vector.tensor_copy(
                                        tpp[jc][:jn, 896 + rs0:896 + rs0 + rw],
                                        a_im[:jn, :rw])

                        # ---- per freq-row-block pipeline ----
                        with tc.tile_pool(name="psC", bufs=1, space="PSUM") as pq:
                            for rb in range(NRB):
                                dT_re = cv.tile([128, 1536], F32R, tag="dTre")
                                dT_im = cv.tile([128, 1536], F32R, tag="dTim")
                                for cs in range(3):
                                    # Fpsf accum
                                    p_re = pq.tile([128, 512], F32, tag="p_re")
                                    p_im = pq.tile([128, 512], F32, tag="p_im")
                                    for jc, (j0, jn) in enumerate(KC_S):
                                        sw = stp.tile([128, 4608], F32R, tag="str")
                                        nc.sync.dma_start(
                                            sw[:jn], ins["wgq_pack"][j0:j0 + jn, :])
                                        t_re = tpp[jc][:jn, rb * 128:rb * 128 + 128]
                                        t_im = tpp[jc][:jn,
                                                       896 + rb * 128:896 + rb * 128 + 128]
                                        q_re = sw[:jn, cs * 512:cs * 512 + 512]
                                        q_im = sw[:jn, 1536 + cs * 512:1536 + cs * 512 + 512]
                                        q_imN = sw[:jn, 3072 + cs * 512:3072 + cs * 512 + 512]
                                        nc.tensor.matmul(p_re[:], t_re, q_re,
                                                         start=(jc == 0), stop=False)
                                        nc.tensor.matmul(p_re[:], t_im, q_imN,
                                                         start=False, stop=(jc == 3))
                                        nc.tensor.matmul(p_im[:], t_re, q_im,
                                                         start=(jc == 0), stop=False)
                                        nc.tensor.matmul(p_im[:], t_im, q_re,
                                                         start=False, stop=(jc == 3))
                                    # Fimg accum
                                    i_re = pq.tile([128, 512], F32, tag="i_re")
                                    i_im = pq.tile([128, 512], F32, tag="i_im")
                                    tw_re = twl[:, rb * 128:rb * 128 + 128]
                                    tw_im = twl[:, 896 + rb * 128:896 + rb * 128 + 128]
                                    tw_imN = twl[:, 1792 + rb * 128:1792 + rb * 128 + 128]
                                    gr = g_re[:, cs * 512:(cs + 1) * 512]
                                    gi = g_im[:, cs * 512:(cs + 1) * 512]
                                    nc.tensor.matmul(i_re[:], tw_re, gr,
                                                     start=True, stop=False)
                                    nc.tensor.matmul(i_re[:], tw_imN, gi,
                                                     start=False, stop=True)
                                    nc.tensor.matmul(i_im[:], tw_re, gi,
                                                     start=True, stop=False)
                                    nc.tensor.matmul(i_im[:], tw_im, gr,
                                                     start=False, stop=True)
                                    # D = Fimg .* Fpsf  (per 512-col seg)
                                    fir = cv.tile([128, 512], F32, tag="fir")
                                    fii = cv.tile([128, 512], F32, tag="fii")
                                    nc.vector.tensor_copy(fir[:], i_re[:])
                                    nc.vector.tensor_copy(fii[:], i_im[:])
                                    t1_ = cv.tile([128, 512], F32, tag="t1")
                                    t2_ = cv.tile([128, 512], F32, tag="t2")
                                    d_re = cv.tile([128, 512], F32, tag="dre")
                                    d_im = cv.tile([128, 512], F32, tag="dim")
                                    nc.vector.tensor_tensor(t1_[:], fir[:], p_re[:],
                                                            ALU.mult)
                                    nc.vector.tensor_tensor(t2_[:], fii[:], p_im[:],
                                                            ALU.mult)
                                    nc.vector.tensor_tensor(d_re[:], t1_[:], t2_[:],
                                                            ALU.subtract)
                                    nc.vector.tensor_tensor(t1_[:], fir[:], p_im[:],
                                                            ALU.mult)
                                    nc.vector.tensor_tensor(t2_[:], fii[:], p_re[:],
                                                            ALU.mult)
                                    nc.vector.tensor_tensor(d_im[:], t1_[:], t2_[:],
                                                            ALU.add)
                                    # transpose D seg -> dT chunks
                                    for b in range(4):
                                        ci = cs * 4 + b
                                        for pl, dst in ((d_re, dT_re), (d_im, dT_im)):
                                            ptr = pq.tile([128, 128], F32, tag="ptrD")
                                            nc.tensor.transpose(
                                                ptr[:], pl[:, b * 128:(b + 1) * 128],
                                                ident[:])
                                            nc.vector.tensor_copy(
                                                dst[:, ci * 128:(ci + 1) * 128], ptr[:])
                                # ---- s1: Y = D^T-contract @ WR^T ----
                                y_re = pq.tile([128, 512], F32, tag="y_re")
                                y_im = pq.tile([128, 512], F32, tag="y_im")
                                for ci in range(12):
                                    sw = stp.tile([128, 4608], F32R, tag="str")
                                    nc.sync.dma_start(
                                        sw[:128, :1536],
                                        ins["wrt_pack"][ci * 128:(ci + 1) * 128, :])
                                    dre_c = dT_re[:, ci * 128:(ci + 1) * 128]
                                    dim_c = dT_im[:, ci * 128:(ci + 1) * 128]
                                    w_re = sw[:128, 0:512]
                                    w_im = sw[:128, 512:1024]
                                    w_imN = sw[:128, 1024:1536]
                                    nc.tensor.matmul(y_re[:], dre_c, w_re,
                                                     start=(ci == 0), stop=False)
                                    nc.tensor.matmul(y_re[:], dim_c, w_imN,
                                                     start=False, stop=(ci == 11))
                                    nc.tensor.matmul(y_im[:], dre_c, w_im,
                                                     start=(ci == 0), stop=False)
                                    nc.tensor.matmul(y_im[:], dim_c, w_re,
                                                     start=False, stop=(ci == 11))
                                # ---- phase: Y' = Y * p_core[r] ----
                                t1_ = cv.tile([128, 512], F32, tag="t1")
                                t2_ = cv.tile([128, 512], F32, tag="t2")
                                yp_re = cv.tile([128, 512], F32R, tag="ypre")
                                yp_im = cv.tile([128, 512], F32R, tag="ypim")
                                pre = scal[:, 8 + 2 * rb:9 + 2 * rb]
                                pim = scal[:, 9 + 2 * rb:10 + 2 * rb]
                                nc.vector.tensor_scalar_mul(t1_[:], y_re[:], pre)
                                nc.vector.tensor_scalar_mul(t2_[:], y_im[:], pim)
                                nc.vector.tensor_tensor(yp_re[:],
                                                        t1_[:], t2_[:], ALU.subtract)
                                nc.vector.tensor_scalar_mul(t1_[:], y_re[:], pim)
                                nc.vector.tensor_scalar_mul(t2_[:], y_im[:], pre)
                                nc.vector.tensor_tensor(yp_im[:],
                                                        t1_[:], t2_[:], ALU.add)
                                # ---- WL accumulation into oacc ----
                                swl = stp.tile([128, 4608], F32R, tag="str")
                                nc.sync.dma_start(
                                    swl[:128, :1024],
                                    ins["wlzt_pack"][rb * 128:(rb + 1) * 128, :])
                                for xt in range(4):
                                    accO = pq.tile([128, 512], F32, tag="accO")
                                    nc.tensor.matmul(
                                        accO[:], swl[:128, xt * 128:xt * 128 + 128],
                                        yp_re[:], start=True, stop=False)
                                    nc.tensor.matmul(
                                        accO[:], swl[:128, 512 + xt * 128:512 + xt * 128 + 128],
                                        yp_im[:], start=False, stop=True)
                                    osl = oacc[:, ch * 2048 + xt * 512:
                                               ch * 2048 + (xt + 1) * 512]
                                    nc.vector.tensor_tensor(osl, osl, accO[:], ALU.add)

            # ---- emit partial outputs as fp16 ----
            for ch in range(3):
                for xt in range(4):
                    oo16 = cp.tile([128, 512], F16, tag="oo16")
                    nc.vector.tensor_copy(
                        oo16[:], oacc[:, ch * 2048 + xt * 512:ch * 2048 + (xt + 1) * 512])
                    nc.sync.dma_start(
                        pout[ch * 512 + xt * 128:ch * 512 + (xt + 1) * 128, :], oo16[:])

    nc.compile()
    _NC["nc"] = nc
    return nc


def _inputs(image, md):
    C = _consts()
    k_arr = C["k_arr"]
    Wp = C["Wp"]
    m2 = np.float32(md * md)
    r_supp = np.sqrt((C["R2supp"] + m2).astype(np.float32)).astype(np.float32)
    rs = np.zeros((512, 512), np.float32)
    rs[:NS, :NS] = r_supp
    in_maps = []
    for core in range(NCORE):
        k0 = 128 * core
        kk = min(128, MX - k0)
        img = np.zeros((384, 1024), ml_dtypes.bfloat16)
        for ch in range(3):
            img[ch * 128:ch * 128 + kk, :MX] = _bf16(image[ch, k0:k0 + kk, :])
        sc = np.zeros((128, 32), np.float32)
        for ch in range(3):
            sc[:, ch] = k_arr[ch]
            sc[:, 3 + ch] = np.float32(k_arr[ch] * md)
        for rb in range(NRB):
            r_ = rb * 128 + np.arange(128)
            ph = Wp[r_ % P, k0 % P]
            sc[:, 8 + 2 * rb] = np.real(ph).astype(np.float32)
            sc[:, 9 + 2 * rb] = np.imag(ph).astype(np.float32)
        in_maps.append({"rs": rs, "img": img, "sc": sc})
    return in_maps


_PJIT_CACHE = {}


def _patch_bass2jax():
    """Cache the pjit callable per-(nc, n_cores) across calls.

    Upstream run_bass_via_pjrt builds a fresh jit closure every call, so each
    launch re-traces + re-compiles (~8s client-side for this NEFF) even though
    the executable is unchanged. This patch keeps upstream semantics (same
    lowering, same shard_map layout, donation) but reuses the compiled
    executable, and converts each output with a single np.asarray.
    """
    from concourse import bass2jax as b2j
    if getattr(b2j, "_mjc_patched", False):
        return
    import jax
    from jax.sharding import Mesh, PartitionSpec
    from jax.experimental.shard_map import shard_map

    def cached_run(nc, in_maps, n_cores):
        import concourse.mybir as _mybir
        b2j.install_neuronx_cc_hook()
        key = (id(nc), n_cores)
        ent = _PJIT_CACHE.get(key)
        if ent is None:
            partition_name = (nc.partition_id_tensor.name
                              if nc.partition_id_tensor else None)
            in_names, out_names, out_avals, zero_shapes = [], [], [], []
            for alloc in nc.m.functions[0].allocations:
                if not isinstance(alloc, _mybir.MemoryLocationSet):
                    continue
                name = alloc.memorylocations[0].name
                if alloc.kind == "ExternalInput":
                    if name != partition_name:
                        in_names.append(name)
                elif alloc.kind == "ExternalOutput":
                    out_names.append(name)
                    shape = tuple(alloc.tensor_shape)
                    dtype = _mybir.dt.np(alloc.dtype)
                    out_avals.append(jax.core.ShapedArray(shape, dtype))
                    zero_shapes.append((shape, dtype))
            n_params = len(in_names)
            n_outs = len(out_avals)
            all_names = list(in_names) + list(out_names)
            if partition_name is not None:
                all_names.append(partition_name)
            donate = tuple(range(n_params, n_params + n_outs))

            def _body(*args):
                operands = list(args)
                if partition_name is not None:
                    operands.append(b2j.partition_id_tensor())
                outs = b2j._bass_exec_p.bind(
                    *operands,
                    out_avals=tuple(out_avals),
                    in_names=tuple(all_names),
                    out_names=tuple(out_names),
                    lowering_input_output_aliases=(),
                    sim_require_finite=True,
                    sim_require_nnan=True,
                    nc=nc,
                )
                return tuple(outs)

            devices = jax.devices()[:n_cores]
            mesh = Mesh(np.asarray(devices), ("core",))
            in_specs = (PartitionSpec("core"),) * (n_params + n_outs)
            out_specs = (PartitionSpec("core"),) * n_outs
            sharded = jax.jit(
                shard_map(_body, mesh=mesh, in_specs=in_specs,
                          out_specs=out_specs, check_rep=False),
                donate_argnums=donate, keep_unused=True)
            ent = (in_names, out_names, out_avals, zero_shapes, sharded)
            _PJIT_CACHE[key] = ent
        in_names, out_names, out_avals, zero_shapes, sharded = ent
        concat_in = [
            np.concatenate([np.asarray(m[name]) for m in in_maps], axis=0)
            for name in in_names
        ]
        concat_zeros = [
            np.zeros((n_cores * s[0], *s[1:]), d) for (s, d) in zero_shapes
        ]
        out_arrs = sharded(*concat_in, *concat_zeros)
        full = [np.asarray(a).reshape(n_cores, *out_avals[i].shape)
                for i, a in enumerate(out_arrs)]
        return [
            {name: full[i][c] for i, name in enumerate(out_names)}
            for c in range(n_cores)
        ]

    b2j.run_bass_via_pjrt = cached_run
    b2j._mjc_patched = True


LAST_TIMES = {}


def kernel(image, depth):
    import time as _time
    image = np.asarray(image, np.float32)
    depth = np.asarray(depth, np.float32)
    try:
        import jax
        import jax.numpy as jnp
        cpu = jax.devices("cpu")[0]
        with jax.default_device(cpu):
            md = np.float32(jax.jit(jnp.mean, backend="cpu")(jax.device_put(depth, cpu)))
    except Exception:
        md = np.float32(np.sum(depth.ravel(), dtype=np.float32) / np.float32(depth.size))

    nc = _build()
    _patch_bass2jax()
    _t0 = _time.time()
    res = run_bass_kernel_spmd(nc, _inputs(image, md), list(range(NCORE)))
    LAST_TIMES["A"] = _time.time() - _t0
    LAST_TIMES["B"] = 0.0

    psums = res.results[0]["psums"]
    Sp = np.float32(np.float32(np.sum(psums[:, :3].astype(np.float64))) + np.float32(1e-7))
    out = np.zeros((3, RES, RES), np.float64)
    for core in range(NCORE):
        po = res.results[core]["pout"].astype(np.float32) * np.float32(2.0 ** 26)
        for ch in range(3):
            out[ch] += po[ch * 512:ch * 512 + RES, :RES].astype(np.float64)
    out = out / np.float64(Sp)
    return np.clip(out, 0.0, 1.0).astype(np.float32)


# revision 13
# speedup vs baseline: 30.1932x; 1.8372x over previous
"""Trainium2 Bass kernel for nn_MjCambrianOptics (depth-invariant PSF + FFT blur).

Single fused SPMD launch on 8 cores (tunnel-payload optimized).

Every core computes the FULL 3-channel PSF on device (replicated: PE time is
cheap, tunnel bytes are not) via the launch-A algebra E = Winv @ Hs @ W1s @ u2
with bit-exact phases fl(k*r), fl(t1*q) (host IEEE sqrt r ships as input;
Cody-Waite reduction + ACT Sin on device). PSF matmuls run in bf16 (incoherent
rounding noise ~2e-3 ≪ 2e-2 tolerance).

The conv side absorbs the reference's psf flip + fftshift row-perm into
statically reordered twiddles: Fpsf[r,c] = Σ P[i,jj] w^{r(1022-perm_s[i])}
w^{c(766-jj)} (only the 511 nonzero psf cols enter). The image is row-sharded
1/8 per core in bf16; Fimg factors as TwL^T @ (imgT @ WgT) with the global row
offset phase w^{r*128*core} deferred to a per-partition complex scale on Y.
Everything downstream is linear in the image, so per-core fp16 partial outputs
sum on the host. Hermitian row truncation (WLz doubling) keeps r-blocks 0..6.

Per-core payload: img slice bf16 0.79MB + r_supp f32 1.05MB + scal; out
fp16 1.57MB. ~28MB total vs ~250MB for the two-launch baseline.
"""
import numpy as np
import ml_dtypes

import concourse.bacc as bacc
import concourse.mybir as mybir
import concourse.tile as tile
from concourse.bass_utils import run_bass_kernel_spmd

F32 = mybir.dt.float32
F32R = mybir.dt.float32r
BF16 = mybir.dt.bfloat16
F16 = mybir.dt.float16
AF = mybir.ActivationFunctionType
ALU = mybir.AluOpType
AX = mybir.AxisListType

MX = 1023
RES = 511
S0 = 256
NS = 511
SENSOR = 0.01
APERTURE = 0.5
WAVELENGTHS = np.array([610e-9, 530e-9, 470e-9], dtype=np.float32)
P = 1536
NRB = 7                 # freq row blocks 0..6 (896 rows >= 769 Hermitian rows)
NCORE = 8

PI = np.float32(np.pi)
TWO_PI = np.float64(2.0) * np.pi
C_RND = float(np.float32(1.5 * 2.0 ** 23))

KC_S = [(0, 128), (128, 128), (256, 128), (384, 127)]        # 511 rows
KC_M = [(i * 128, 128) for i in range(7)] + [(896, 127)]     # 1023 rows


def _r32c(x):
    """Round ndarray to f32r (12-bit significand), RNE — matches tensor_copy."""
    f = np.ascontiguousarray(x, np.float32)
    b = f.view(np.uint32).astype(np.uint64)
    low = b & 0xFFF
    b2 = b & ~np.uint64(0xFFF)
    up = (low > 0x800) | ((low == 0x800) & (((b2 >> 12) & 1) == 1))
    b2 = b2 + np.where(up, np.uint64(0x1000), np.uint64(0))
    return b2.astype(np.uint32).view(np.float32).reshape(f.shape)


def _bf16(x):
    return np.ascontiguousarray(x, np.float32).astype(ml_dtypes.bfloat16)


def _splitb(x64, keep):
    f = np.float32(x64)
    mask = np.uint32(0xFFFFFFFF ^ ((1 << (24 - keep)) - 1))
    bits = np.uint32(int(f.view(np.uint32)) & int(mask))
    return bits.view(np.float32)


P1 = _splitb(TWO_PI, 11)
P2 = _splitb(TWO_PI - np.float64(P1), 10)
P3 = np.float32(TWO_PI - np.float64(P1) - np.float64(P2))
P1H = np.float32(np.float64(P1) * 256.0)
P2H = np.float32(np.float64(P2) * 256.0)

_CONSTS = {}


def _consts():
    if _CONSTS:
        return _CONSTS
    dx = SENSOR / MX
    Lx = dx * MX
    x1 = np.linspace(-Lx / 2, Lx / 2, MX, dtype=np.float32)
    X1, Y1 = np.meshgrid(x1, x1, indexing="ij")
    fx = np.linspace(-1.0 / (2 * dx), 1.0 / (2 * dx), MX, dtype=np.float32)
    FX, FY = np.meshgrid(fx, fx, indexing="ij")
    ar = (Lx / 2.0) * APERTURE
    A = (np.sqrt(X1 ** 2 + Y1 ** 2) / np.float32(ar + 1e-7) <= 1.0).astype(np.float32)
    lam = WAVELENGTHS
    k_arr = (np.float32(2.0) * np.float32(np.pi) / lam).astype(np.float32)
    q = np.empty((3, MX, MX), np.float32)
    for c in range(3):
        a_ = (lam[c] * FX).astype(np.float32)
        b_ = (lam[c] * FY).astype(np.float32)
        s_ = ((np.float32(1.0) - (a_ * a_).astype(np.float32)).astype(np.float32)
              - (b_ * b_).astype(np.float32)).astype(np.float32)
        q[c] = np.sqrt(s_).astype(np.float32)
    R2 = ((X1 * X1).astype(np.float32) + (Y1 * Y1).astype(np.float32)).astype(np.float32)

    jk = np.arange(MX)
    W = np.exp(-2j * np.pi * np.outer(jk, jk) / MX)
    Winv = np.conj(W) / MX
    perm_s = (jk - MX // 2) % MX
    perm_si = (jk + MX // 2) % MX
    WS_s = W[:, perm_si][:, S0:S0 + NS]        # [1023 x 511]
    qs_sh = q[:, perm_s][:, :, perm_s]         # Hs = exp(i t1 qs_sh)

    jP = np.arange(P)
    Wp = np.exp(-2j * np.pi * np.outer(jP, jP) / P)
    Winvp = np.conj(Wp) / P
    selr = 767 + np.arange(RES)
    WLm = Winvp[selr, :769].copy()
    WLm[:, 1:768] *= 2.0
    WLz = np.zeros((RES, NRB * 128), np.complex128)
    WLz[:, :769] = WLm
    WR = Winvp[selr, :]                        # [511 x 1536]

    kvec = 1022 - perm_s                       # psf row exponent map
    mvec = 766 - np.arange(NS)                 # psf col exponent map

    C = {}
    C["k_arr"] = k_arr
    C["R2supp"] = R2[S0:S0 + NS, S0:S0 + NS]
    amask = np.zeros((512, 512), np.float32)
    amask[:NS, :NS] = A[S0:S0 + NS, S0:S0 + NS]
    C["amask"] = amask
    for c in range(3):
        C[f"qs{c}"] = np.ascontiguousarray(qs_sh[c])

    w1p = np.zeros((NS, 2048), np.float32)
    w1p[:, :MX] = np.real(WS_s).T
    w1p[:, 1024:1024 + MX] = np.imag(WS_s).T
    C["w1s_pack"] = _bf16(w1p)                 # [511 x 2048] bf16 lhsT step1
    wvp = np.zeros((MX, 2048), np.float32)
    wvp[:, :MX] = np.real(Winv)                # symmetric
    wvp[:, 1024:1024 + MX] = np.imag(Winv)
    C["winv_pack"] = _bf16(wvp)

    wgpt = np.zeros((MX, 1792), np.float64)    # [i x (re 896 | im 896)]
    WgP = Wp[:896, :][:, kvec]                 # [896 r x 1023 i]
    wgpt[:, :896] = np.real(WgP).T
    wgpt[:, 896:1792] = np.imag(WgP).T
    C["wgpt_pack"] = _bf16(wgpt)

    wgq = np.zeros((NS, 4608), np.float64)     # [jj x (re|im|imN 1536)]
    WgQ = Wp[mvec, :]                          # [511 jj x 1536 c]
    wgq[:, :1536] = np.real(WgQ)
    wgq[:, 1536:3072] = np.imag(WgQ)
    wgq[:, 3072:4608] = -np.imag(WgQ)
    C["wgq_pack"] = _bf16(wgq)

    wgt = np.zeros((MX, 3072), np.float64)     # [m x (re|im 1536)] w^{cm}
    WgT = Wp[:, :MX].T
    wgt[:, :1536] = np.real(WgT)
    wgt[:, 1536:3072] = np.imag(WgT)
    C["wgt_pack"] = _bf16(wgt)

    twlf = np.zeros((MX, 1792), np.float64)    # [k x (re|im 896)] w^{r k}
    TwF = Wp[:896, :MX].T
    twlf[:, :896] = np.real(TwF)
    twlf[:, 896:1792] = np.imag(TwF)
    C["twlf_pack"] = _bf16(twlf)

    wrt = np.zeros((P, 1536), np.float64)      # [c x (re|im|imN 512)] WR^T
    wrt[:, :RES] = np.real(WR).T
    wrt[:, 512:512 + RES] = np.imag(WR).T
    wrt[:, 1024:1024 + RES] = -np.imag(WR).T
    C["wrt_pack"] = _bf16(wrt)

    wlz = np.zeros((NRB * 128, 1024), np.float64)  # [r x (re 512 | imN 512)]
    wlz[:, :RES] = np.real(WLz).T * (2.0 ** -26)   # fp16-range scale, exact
    wlz[:, 512:512 + RES] = -np.imag(WLz).T * (2.0 ** -26)
    C["wlzt_pack"] = _bf16(wlz)

    C["ident"] = np.eye(128, dtype=np.float32)
    C["Wp"] = Wp
    _CONSTS.update(C)
    return _CONSTS


# ---------------------------------------------------------------------------
_NC = {}


def _build():
    if "nc" in _NC:
        return _NC["nc"]
    nc = bacc.Bacc("TRN2", target_bir_lowering=False, debug=False)
    C = _consts()
    ins = {}
    for nm in ["wgpt_pack", "wgq_pack", "wgt_pack", "twlf_pack", "wrt_pack",
               "wlzt_pack", "w1s_pack", "winv_pack"]:
        ins[nm] = nc.inline_tensor(C[nm], nm).ap()
    for c in range(3):
        ins[f"qs{c}"] = nc.inline_tensor(C[f"qs{c}"], f"qs{c}").ap()
    ins["amask"] = nc.inline_tensor(C["amask"], "amask").ap()
    ins["ident"] = nc.inline_tensor(C["ident"], "ident").ap()
    ins["rs"] = nc.dram_tensor("rs", [512, 512], F32, kind="ExternalInput").ap()
    ins["img"] = nc.dram_tensor("img", [3072, 1024], BF16, kind="ExternalInput").ap()
    ins["sc"] = nc.dram_tensor("sc", [128, 32], F32, kind="ExternalInput").ap()
    pout = nc.dram_tensor("pout", [1536, 512], F16, kind="ExternalOutput").ap()
    psums = nc.dram_tensor("psums", [128, 4], F32, kind="ExternalOutput").ap()

    with tile.TileContext(nc) as tc:
        with (
            tc.tile_pool(name="cst", bufs=1) as cp,
            tc.tile_pool(name="acc", bufs=1) as ao,
        ):
            scal = cp.tile([128, 32], F32, tag="scal")
            nc.sync.dma_start(scal[:], ins["sc"][:])
            ident = cp.tile([128, 128], F32, tag="ident")
            nc.sync.dma_start(ident[:], ins["ident"][:])
            lrs = cp.tile([128, 4 * 512], F32, tag="lrs")
            lmsk = cp.tile([128, 4 * 512], F32, tag="lmsk")
            for ci, (r0, rn) in enumerate(KC_S):
                nc.sync.dma_start(lrs[:rn, ci * 512:(ci + 1) * 512],
                                  ins["rs"][r0:r0 + rn, :])
                nc.sync.dma_start(lmsk[:rn, ci * 512:(ci + 1) * 512],
                                  ins["amask"][r0:r0 + rn, :])
            oacc = ao.tile([128, 3 * 2048], F32, tag="oacc")
            nc.vector.memset(oacc[:], 0.0)

            def trig_pair(dst_cos, dst_sin, base_ap, t_col, rows, w,
                          tg, mask_ap=None):
                th = tg.tile([128, MX], F32, tag="th")
                nc.vector.tensor_scalar_mul(th[:rows, :w], base_ap,
                                            scal[:rows, t_col:t_col + 1])
                f = tg.tile([128, MX], F32, tag="f")
                nc.vector.tensor_scalar(f[:rows, :w], th[:rows, :w],
                                        float(np.float32(1.0 / TWO_PI)), C_RND,
                                        ALU.mult, ALU.add)
                nc.vector.tensor_scalar_sub(f[:rows, :w], f[:rows, :w], C_RND)
                g = tg.tile([128, MX], F32, tag="g")
                nc.vector.tensor_scalar(g[:rows, :w], f[:rows, :w],
                                        float(np.float32(1.0 / 256.0)), C_RND,
                                        ALU.mult, ALU.add)
                nc.vector.tensor_scalar_sub(g[:rows, :w], g[:rows, :w], C_RND)
                nl = tg.tile([128, MX], F32, tag="nl")
                nc.vector.cody_waite_cascade(nl[:rows, :w], f[:rows, :w],
                                             g[:rows, :w], 256.0, 0.0, 0.0)
                y = tg.tile([128, MX], F32, tag="y")
                nc.vector.cody_waite_cascade(y[:rows, :w], th[:rows, :w],
                                             g[:rows, :w], float(P1H), float(P2H), 0.0)
                nc.vector.cody_waite_cascade(y[:rows, :w], y[:rows, :w],
                                             nl[:rows, :w], float(P1), float(P2), 0.0)
                nc.vector.cody_waite_cascade(y[:rows, :w], y[:rows, :w],
                                             f[:rows, :w], float(P3), 0.0, 0.0)
                yw = tg.tile([128, MX], F32, tag="yw")
                nc.vector.add_range_wrap(yw[:rows, :w], y[:rows, :w], 0.0,
                                         float(PI), float(np.float32(2 * np.pi)))
                yc = tg.tile([128, MX], F32, tag="yc")
                nc.vector.add_range_wrap(yc[:rows, :w], y[:rows, :w],
                                         float(np.float32(PI / 2)), float(PI),
                                         float(np.float32(2 * np.pi)))
                if mask_ap is None:
                    nc.scalar.activation(dst_sin, yw[:rows, :w], AF.Sin)
                    nc.scalar.activation(dst_cos, yc[:rows, :w], AF.Sin)
                else:
                    sn = tg.tile([128, 256], F32, tag="sn")
                    cn = tg.tile([128, 256], F32, tag="cn")
                    nc.scalar.activation(sn[:rows, :w], yw[:rows, :w], AF.Sin)
                    nc.scalar.activation(cn[:rows, :w], yc[:rows, :w], AF.Sin)
                    nc.vector.tensor_tensor(dst_sin, sn[:rows, :w], mask_ap, ALU.mult)
                    nc.vector.tensor_tensor(dst_cos, cn[:rows, :w], mask_ap, ALU.mult)

            for ch in range(3):
                with tc.tile_pool(name="chan", bufs=1) as chp:
                    ps_t = chp.tile([128, 8 * 512], BF16, tag="ps_t")
                    nc.vector.memset(ps_t[:], 0.0)

                    # ======== PSF phase: E = Winv @ Hs @ W1s @ u2 ========
                    with (
                        tc.tile_pool(name="psfp", bufs=1) as pf,
                        tc.tile_pool(name="trg", bufs=1) as tg,
                        tc.tile_pool(name="wbf", bufs=2) as wb,
                        tc.tile_pool(name="psA", bufs=2, space="PSUM") as pp,
                    ):
                        # Hs trig (resident bf16, 8 chunks)
                        hs = {}
                        for ci, (r0, rn) in enumerate(KC_M):
                            qsl = tg.tile([128, MX], F32, tag="qsl")
                            nc.sync.dma_start(qsl[:rn], ins[f"qs{ch}"][r0:r0 + rn, :])
                            hre = pf.tile([128, MX], BF16, tag=f"hre{ci}")
                            him = pf.tile([128, MX], BF16, tag=f"him{ci}")
                            trig_pair(hre[:rn], him[:rn], qsl[:rn], 3 + ch,
                                      rn, MX, tg)
                            hs[ci] = (hre, him)

                        # u2 trig: per chunk [h0re 256|h0im 256|h1re|h1im]
                        u2_m = pf.tile([128, 4 * 1024], BF16, tag="u2m")
                        u2_n = pf.tile([128, 4 * 512], BF16, tag="u2n")
                        nc.vector.memset(u2_m[:], 0.0)
                        for ci, (r0, rn) in enumerate(KC_S):
                            for h in range(2):
                                o = ci * 1024 + h * 512
                                base = lrs[:rn, ci * 512 + h * 256:
                                           ci * 512 + h * 256 + 256]
                                mk = lmsk[:rn, ci * 512 + h * 256:
                                          ci * 512 + h * 256 + 256]
                                trig_pair(u2_m[:rn, o:o + 256],
                                          u2_m[:rn, o + 256:o + 512],
                                          base, ch, rn, 256, tg, mask_ap=mk)
                            nc.vector.tensor_scalar_mul(
                                u2_n[:rn, ci * 512:ci * 512 + 256],
                                u2_m[:rn, ci * 1024 + 256:ci * 1024 + 512], -1.0)
                            nc.vector.tensor_scalar_mul(
                                u2_n[:rn, ci * 512 + 256:ci * 512 + 512],
                                u2_m[:rn, ci * 1024 + 768:ci * 1024 + 1024], -1.0)

                        def cmm(acc, lre, lim, m_full, m_re, m_im_neg, first, last):
                            nc.tensor.matmul(acc[:, 0:512], lre, m_full,
                                             start=first, stop=False)
                            nc.tensor.matmul(acc[:, 0:256], lim, m_im_neg,
                                             start=False, stop=False)
                            nc.tensor.matmul(acc[:, 256:512], lim, m_re,
                                             start=False, stop=last)

                        for h in range(2):
                            # ---- step1: X1 = W1s @ u2(half) ----
                            x1_m = pf.tile([128, 8 * 512], BF16, tag="x1m")
                            x1_n = pf.tile([128, 8 * 256], BF16, tag="x1n")
                            for mi, (m0, mn) in enumerate(KC_M):
                                acc = pp.tile([128, 512], F32, tag="accA")
                                for ci, (r0, rn) in enumerate(KC_S):
                                    lw = wb.tile([128, 2048], BF16, tag="wbf")
                                    nc.sync.dma_start(
                                        lw[:rn, :mn],
                                        ins["w1s_pack"][r0:r0 + rn, m0:m0 + mn])
                                    nc.sync.dma_start(
                                        lw[:rn, 1024:1024 + mn],
                                        ins["w1s_pack"][r0:r0 + rn,
                                                        1024 + m0:1024 + m0 + mn])
                                    o = ci * 1024 + h * 512
                                    cmm(acc[:mn], lw[:rn, :mn],
                                        lw[:rn, 1024:1024 + mn],
                                        u2_m[:rn, o:o + 512],
                                        u2_m[:rn, o:o + 256],
                                        u2_n[:rn, ci * 512 + h * 256:
                                             ci * 512 + h * 256 + 256],
                                        ci == 0, ci == 3)
                                o = mi * 512
                                nc.vector.tensor_copy(x1_m[:mn, o:o + 512], acc[:mn])
                                nc.vector.tensor_scalar_mul(
                                    x1_n[:mn, mi * 256:(mi + 1) * 256],
                                    acc[:mn, 256:512], -1.0)

                            # ---- step2: X2 = Hs @ X1 ----
                            x2_m = pf.tile([128, 8 * 512], BF16, tag="x2m")
                            x2_n = pf.tile([128, 8 * 256], BF16, tag="x2n")
                            for mi, (m0, mn) in enumerate(KC_M):
                                acc = pp.tile([128, 512], F32, tag="accB")
                                for ci, (r0, rn) in enumerate(KC_M):
                                    hre, him = hs[ci]
                                    o = ci * 512
                                    cmm(acc[:mn], hre[:rn, m0:m0 + mn],
                                        him[:rn, m0:m0 + mn],
                                        x1_m[:rn, o:o + 512],
                                        x1_m[:rn, o:o + 256],
                                        x1_n[:rn, ci * 256:(ci + 1) * 256],
                                        ci == 0, ci == 7)
                                o = mi * 512
                                nc.vector.tensor_copy(x2_m[:mn, o:o + 512], acc[:mn])
                                nc.vector.tensor_scalar_mul(
                                    x2_n[:mn, mi * 256:(mi + 1) * 256],
                                    acc[:mn, 256:512], -1.0)

                            # ---- step3: E = Winv @ X2; ps_t += |E|^2 ----
                            for mi, (m0, mn) in enumerate(KC_M):
                                acc = pp.tile([128, 512], F32, tag="accC")
                                for ci, (r0, rn) in enumerate(KC_M):
                                    lw = wb.tile([128, 2048], BF16, tag="wbf")
                                    nc.sync.dma_start(
                                        lw[:rn, :mn],
                                        ins["winv_pack"][r0:r0 + rn, m0:m0 + mn])
                                    nc.sync.dma_start(
                                        lw[:rn, 1024:1024 + mn],
                                        ins["winv_pack"][r0:r0 + rn,
                                                         1024 + m0:1024 + m0 + mn])
                                    o = ci * 512
                                    cmm(acc[:mn], lw[:rn, :mn],
                                        lw[:rn, 1024:1024 + mn],
                                        x2_m[:rn, o:o + 512],
                                        x2_m[:rn, o:o + 256],
                                        x2_n[:rn, ci * 256:(ci + 1) * 256],
                                        ci == 0, ci == 7)
                                e_sb = tg.tile([128, 512], F32, tag="esb")
                                nc.vector.tensor_copy(e_sb[:mn], acc[:mn])
                                sq = tg.tile([128, 256], F32, tag="sq")
                                nc.vector.tensor_tensor(sq[:mn], e_sb[:mn, 0:256],
                                                        e_sb[:mn, 0:256], ALU.mult)
                                sq2 = tg.tile([128, 256], F32, tag="sq2")
                                nc.vector.tensor_tensor(sq2[:mn], e_sb[:mn, 256:512],
                                                        e_sb[:mn, 256:512], ALU.mult)
                                nc.vector.tensor_tensor(
                                    ps_t[:mn, mi * 512 + h * 256:
                                         mi * 512 + h * 256 + 256],
                                    sq[:mn], sq2[:mn], ALU.add)

                    # psf channel sum -> psums[:, ch]
                    rsum = cp.tile([128, 4], F32, tag="rsum")
                    rtmp = cp.tile([128, 8 * 512], F32, tag="rtmp")
                    nc.vector.tensor_copy(rtmp[:], ps_t[:])
                    nc.vector.tensor_reduce(rsum[:, ch:ch + 1],
                                            rtmp[:], AX.X, ALU.add)
                    nc.sync.dma_start(psums[:, ch:ch + 1], rsum[:, ch:ch + 1])

                    # ======== CONV phase ========
                    with (
                        tc.tile_pool(name="convp", bufs=1) as cv,
                        tc.tile_pool(name="str", bufs=2) as stp,
                    ):
                        # ---- T[m, r] = sum_k img[k, m] w^{rk} (col-DFT) ----
                        imt = [cv.tile([128, 1024], BF16, tag=f"imt{b}",
                                       name=f"imt{b}") for b in range(8)]
                        for b, (k0, kn) in enumerate(KC_M):
                            nc.sync.dma_start(
                                imt[b][:kn],
                                ins["img"][ch * 1024 + k0:ch * 1024 + k0 + kn, :])
                        tpi = [cv.tile([128, 2688], BF16, tag=f"tpi{mc}",
                                       name=f"tpi{mc}") for mc in range(8)]
                        with tc.tile_pool(name="psI", bufs=2, space="PSUM") as pg:
                            for rs0, rw in ((0, 512), (512, 384)):
                                for mc, (m0, mn) in enumerate(KC_M):
                                    a_re = pg.tile([128, 512], F32, tag="ti_re")
                                    a_im = pg.tile([128, 512], F32, tag="ti_im")
                                    for kc, (k0, kn) in enumerate(KC_M):
                                        sw = stp.tile([128, 4608], BF16, tag="str")
                                        nc.sync.dma_start(
                                            sw[:kn, :rw],
                                            ins["twlf_pack"][k0:k0 + kn, rs0:rs0 + rw])
                                        nc.sync.dma_start(
                                            sw[:kn, 1024:1024 + rw],
                                            ins["twlf_pack"][k0:k0 + kn,
                                                             896 + rs0:896 + rs0 + rw])
                                        lhs = imt[kc][:kn, m0:m0 + mn]
                                        nc.tensor.matmul(a_re[:mn, :rw], lhs,
                                                         sw[:kn, :rw],
                                                         start=(kc == 0), stop=(kc == 7))
                                        nc.tensor.matmul(a_im[:mn, :rw], lhs,
                                                         sw[:kn, 1024:1024 + rw],
                                                         start=(kc == 0), stop=(kc == 7))
                                    nc.vector.tensor_copy(
                                        tpi[mc][:mn, rs0:rs0 + rw], a_re[:mn, :rw])
                                    nc.vector.tensor_copy(
                                        tpi[mc][:mn, 896 + rs0:896 + rs0 + rw],
                                        a_im[:mn, :rw])
                                    nc.vector.tensor_scalar_mul(
                                        tpi[mc][:mn, 1792 + rs0:1792 + rs0 + rw],
                                        a_im[:mn, :rw], -1.0)

                        # ---- tmpT_p[jj, r] = sum_i P[i,jj] WgP[r,i] ----
                        tpp = [cv.tile([128, 1792], BF16, tag=f"tpp{j}", name=f"tpp{j}")
                               for j in range(4)]
                        with tc.tile_pool(name="psT", bufs=2, space="PSUM") as pg:
                            for rs0, rw in ((0, 512), (512, 384)):
                                for jc, (j0, jn) in enumerate(KC_S):
                                    a_re = pg.tile([128, 512], F32, tag="tp_re")
                                    a_im = pg.tile([128, 512], F32, tag="tp_im")
                                    for ic, (i0, icn) in enumerate(KC_M):
                                        sw = stp.tile([128, 4608], BF16, tag="str")
                                        nc.sync.dma_start(
                                            sw[:icn, :rw],
                                            ins["wgpt_pack"][i0:i0 + icn, rs0:rs0 + rw])
                                        nc.sync.dma_start(
                                            sw[:icn, 1024:1024 + rw],
                                            ins["wgpt_pack"][i0:i0 + icn,
                                                             896 + rs0:896 + rs0 + rw])
                                        lhs = ps_t[:icn, ic * 512 + j0:ic * 512 + j0 + jn]
                                        nc.tensor.matmul(a_re[:jn, :rw], lhs,
                                                         sw[:icn, :rw],
                                                         start=(ic == 0), stop=(ic == 7))
                                        nc.tensor.matmul(a_im[:jn, :rw], lhs,
                                                         sw[:icn, 1024:1024 + rw],
                                                         start=(ic == 0), stop=(ic == 7))
                                    nc.vector.tensor_copy(
                                        tpp[jc][:jn, rs0:rs0 + rw], a_re[:jn, :rw])
                                    nc.vector.tensor_copy(
                                        tpp[jc][:jn, 896 + rs0:896 + rs0 + rw],
                                        a_im[:jn, :rw])

                        # ---- per freq-row-block pipeline ----
                        with tc.tile_pool(name="psC", bufs=1, space="PSUM") as pq:
                            for rb in range(NRB):
                                dT_re = cv.tile([128, 1536], BF16, tag="dTre")
                                dT_im = cv.tile([128, 1536], BF16, tag="dTim")
                                for cs in range(3):
                                    # Fpsf accum
                                    p_re = pq.tile([128, 512], F32, tag="p_re")
                                    p_im = pq.tile([128, 512], F32, tag="p_im")
                                    for jc, (j0, jn) in enumerate(KC_S):
                                        sw = stp.tile([128, 4608], BF16, tag="str")
                                        nc.sync.dma_start(
                                            sw[:jn], ins["wgq_pack"][j0:j0 + jn, :])
                                        t_re = tpp[jc][:jn, rb * 128:rb * 128 + 128]
                                        t_im = tpp[jc][:jn,
                                                       896 + rb * 128:896 + rb * 128 + 128]
                                        q_re = sw[:jn, cs * 512:cs * 512 + 512]
                                        q_im = sw[:jn, 1536 + cs * 512:1536 + cs * 512 + 512]
                                        q_imN = sw[:jn, 3072 + cs * 512:3072 + cs * 512 + 512]
                                        nc.tensor.matmul(p_re[:], t_re, q_re,
                                                         start=(jc == 0), stop=False)
                                        nc.tensor.matmul(p_re[:], t_im, q_imN,
                                                         start=False, stop=(jc == 3))
                                        nc.tensor.matmul(p_im[:], t_re, q_im,
                                                         start=(jc == 0), stop=False)
                                        nc.tensor.matmul(p_im[:], t_im, q_re,
                                                         start=False, stop=(jc == 3))
                                    # Fimg accum: sum_m T[m,r] WgT[m,c]
                                    i_re = pq.tile([128, 512], F32, tag="i_re")
                                    i_im = pq.tile([128, 512], F32, tag="i_im")
                                    for mc, (m0, mn) in enumerate(KC_M):
                                        sw = stp.tile([128, 4608], BF16, tag="str")
                                        nc.sync.dma_start(
                                            sw[:mn, :512],
                                            ins["wgt_pack"][m0:m0 + mn,
                                                            cs * 512:cs * 512 + 512])
                                        nc.sync.dma_start(
                                            sw[:mn, 1024:1536],
                                            ins["wgt_pack"][m0:m0 + mn,
                                                            1536 + cs * 512:1536 + cs * 512 + 512])
                                        t_re = tpi[mc][:mn, rb * 128:rb * 128 + 128]
                                        t_im = tpi[mc][:mn,
                                                       896 + rb * 128:896 + rb * 128 + 128]
                                        t_imN = tpi[mc][:mn,
                                                        1792 + rb * 128:1792 + rb * 128 + 128]
                                        w_re = sw[:mn, :512]
                                        w_im = sw[:mn, 1024:1536]
                                        nc.tensor.matmul(i_re[:], t_re, w_re,
                                                         start=(mc == 0), stop=False)
                                        nc.tensor.matmul(i_re[:], t_imN, w_im,
                                                         start=False, stop=(mc == 7))
                                        nc.tensor.matmul(i_im[:], t_re, w_im,
                                                         start=(mc == 0), stop=False)
                                        nc.tensor.matmul(i_im[:], t_im, w_re,
                                                         start=False, stop=(mc == 7))
                                    # D = Fimg .* Fpsf  (per 512-col seg)
                                    fir = cv.tile([128, 512], F32, tag="fir")
                                    fii = cv.tile([128, 512], F32, tag="fii")
                                    nc.vector.tensor_copy(fir[:], i_re[:])
                                    nc.vector.tensor_copy(fii[:], i_im[:])
                                    t1_ = cv.tile([128, 512], F32, tag="t1")
                                    t2_ = cv.tile([128, 512], F32, tag="t2")
                                    d_re = cv.tile([128, 512], F32, tag="dre")
                                    d_im = cv.tile([128, 512], F32, tag="dim")
                                    nc.vector.tensor_tensor(t1_[:], fir[:], p_re[:],
                                                            ALU.mult)
                                    nc.vector.tensor_tensor(t2_[:], fii[:], p_im[:],
                                                            ALU.mult)
                                    nc.vector.tensor_tensor(d_re[:], t1_[:], t2_[:],
                                                            ALU.subtract)
                                    nc.vector.tensor_tensor(t1_[:], fir[:], p_im[:],
                                                            ALU.mult)
                                    nc.vector.tensor_tensor(t2_[:], fii[:], p_re[:],
                                                            ALU.mult)
                                    nc.vector.tensor_tensor(d_im[:], t1_[:], t2_[:],
                                                            ALU.add)
                                    # transpose D seg -> dT chunks
                                    for b in range(4):
                                        ci = cs * 4 + b
                                        for pl, dst in ((d_re, dT_re), (d_im, dT_im)):
                                            ptr = pq.tile([128, 128], F32, tag="ptrD")
                                            nc.tensor.transpose(
                                                ptr[:], pl[:, b * 128:(b + 1) * 128],
                                                ident[:])
                                            nc.vector.tensor_copy(
                                                dst[:, ci * 128:(ci + 1) * 128], ptr[:])
                                # ---- s1: Y = D^T-contract @ WR^T ----
                                y_re = pq.tile([128, 512], F32, tag="y_re")
                                y_im = pq.tile([128, 512], F32, tag="y_im")
                                for ci in range(12):
                                    sw = stp.tile([128, 4608], BF16, tag="str")
                                    nc.sync.dma_start(
                                        sw[:128, :1536],
                                        ins["wrt_pack"][ci * 128:(ci + 1) * 128, :])
                                    dre_c = dT_re[:, ci * 128:(ci + 1) * 128]
                                    dim_c = dT_im[:, ci * 128:(ci + 1) * 128]
                                    w_re = sw[:128, 0:512]
                                    w_im = sw[:128, 512:1024]
                                    w_imN = sw[:128, 1024:1536]
                                    nc.tensor.matmul(y_re[:], dre_c, w_re,
                                                     start=(ci == 0), stop=False)
                                    nc.tensor.matmul(y_re[:], dim_c, w_imN,
                                                     start=False, stop=(ci == 11))
                                    nc.tensor.matmul(y_im[:], dre_c, w_im,
                                                     start=(ci == 0), stop=False)
                                    nc.tensor.matmul(y_im[:], dim_c, w_re,
                                                     start=False, stop=(ci == 11))
                                yp_re = cv.tile([128, 512], BF16, tag="ypre")
                                yp_im = cv.tile([128, 512], BF16, tag="ypim")
                                nc.vector.tensor_copy(yp_re[:], y_re[:])
                                nc.vector.tensor_copy(yp_im[:], y_im[:])
                                # ---- WL accumulation into oacc ----
                                swl = stp.tile([128, 4608], BF16, tag="str")
                                nc.sync.dma_start(
                                    swl[:128, :1024],
                                    ins["wlzt_pack"][rb * 128:(rb + 1) * 128, :])
                                for xt in range(4):
                                    accO = pq.tile([128, 512], F32, tag="accO")
                                    nc.tensor.matmul(
                                        accO[:], swl[:128, xt * 128:xt * 128 + 128],
                                        yp_re[:], start=True, stop=False)
                                    nc.tensor.matmul(
                                        accO[:], swl[:128, 512 + xt * 128:512 + xt * 128 + 128],
                                        yp_im[:], start=False, stop=True)
                                    osl = oacc[:, ch * 2048 + xt * 512:
                                               ch * 2048 + (xt + 1) * 512]
                                    nc.vector.tensor_tensor(osl, osl, accO[:], ALU.add)

            # ---- emit partial outputs as fp16 ----
            for ch in range(3):
                for xt in range(4):
                    oo16 = cp.tile([128, 512], F16, tag="oo16")
                    nc.vector.tensor_copy(
                        oo16[:], oacc[:, ch * 2048 + xt * 512:ch * 2048 + (xt + 1) * 512])
                    nc.sync.dma_start(
                        pout[ch * 512 + xt * 128:ch * 512 + (xt + 1) * 128, :], oo16[:])

    nc.compile()
    _NC["nc"] = nc
    return nc


def _inputs(image, md):
    C = _consts()
    k_arr = C["k_arr"]
    m2 = np.float32(md * md)
    r_supp = np.sqrt((C["R2supp"] + m2).astype(np.float32)).astype(np.float32)
    rs = np.zeros((512, 512), np.float32)
    rs[:NS, :NS] = r_supp
    img = np.zeros((3072, 1024), ml_dtypes.bfloat16)
    for ch in range(3):
        img[ch * 1024:ch * 1024 + MX, :MX] = _bf16(image[ch])
    sc = np.zeros((128, 32), np.float32)
    for ch in range(3):
        sc[:, ch] = k_arr[ch]
        sc[:, 3 + ch] = np.float32(k_arr[ch] * md)
    return [{"rs": rs, "img": img, "sc": sc}]


_PJIT_CACHE = {}


def _patch_bass2jax():
    """Cache the pjit callable per-(nc, n_cores) across calls.

    Upstream run_bass_via_pjrt builds a fresh jit closure every call, so each
    launch re-traces + re-compiles (~8s client-side for this NEFF) even though
    the executable is unchanged. This patch keeps upstream semantics (same
    lowering, same shard_map layout, donation) but reuses the compiled
    executable, and converts each output with a single np.asarray.
    """
    from concourse import bass2jax as b2j
    if getattr(b2j, "_mjc_patched", False):
        return
    import jax
    from jax.sharding import Mesh, PartitionSpec
    from jax.experimental.shard_map import shard_map

    def cached_run(nc, in_maps, n_cores):
        import concourse.mybir as _mybir
        b2j.install_neuronx_cc_hook()
        key = (id(nc), n_cores)
        ent = _PJIT_CACHE.get(key)
        if ent is None:
            partition_name = (nc.partition_id_tensor.name
                              if nc.partition_id_tensor else None)
            in_names, out_names, out_avals, zero_shapes = [], [], [], []
            for alloc in nc.m.functions[0].allocations:
                if not isinstance(alloc, _mybir.MemoryLocationSet):
                    continue
                name = alloc.memorylocations[0].name
                if alloc.kind == "ExternalInput":
                    if name != partition_name:
                        in_names.append(name)
                elif alloc.kind == "ExternalOutput":
                    out_names.append(name)
                    shape = tuple(alloc.tensor_shape)
                    dtype = _mybir.dt.np(alloc.dtype)
                    out_avals.append(jax.core.ShapedArray(shape, dtype))
                    zero_shapes.append((shape, dtype))
            n_params = len(in_names)
            n_outs = len(out_avals)
            all_names = list(in_names) + list(out_names)
            if partition_name is not None:
                all_names.append(partition_name)
            donate = tuple(range(n_params, n_params + n_outs))

            def _body(*args):
                operands = list(args)
                if partition_name is not None:
                    operands.append(b2j.partition_id_tensor())
                outs = b2j._bass_exec_p.bind(
                    *operands,
                    out_avals=tuple(out_avals),
                    in_names=tuple(all_names),
                    out_names=tuple(out_names),
                    lowering_input_output_aliases=(),
                    sim_require_finite=True,
                    sim_require_nnan=True,
                    nc=nc,
                )
                return tuple(outs)

            devices = jax.devices()[:n_cores]
            mesh = Mesh(np.asarray(devices), ("core",))
            in_specs = (PartitionSpec("core"),) * (n_params + n_outs)
            out_specs = (PartitionSpec("core"),) * n_outs
            sharded = jax.jit(
                shard_map(_body, mesh=mesh, in_specs=in_specs,
                          out_specs=out_specs, check_rep=False),
                donate_argnums=donate, keep_unused=True)
            ent = (in_names, out_names, out_avals, zero_shapes, sharded)
            _PJIT_CACHE[key] = ent
        in_names, out_names, out_avals, zero_shapes, sharded = ent
        concat_in = [
            np.concatenate([np.asarray(m[name]) for m in in_maps], axis=0)
            for name in in_names
        ]
        concat_zeros = [
            np.zeros((n_cores * s[0], *s[1:]), d) for (s, d) in zero_shapes
        ]
        out_arrs = sharded(*concat_in, *concat_zeros)
        full = [np.asarray(a).reshape(n_cores, *out_avals[i].shape)
                for i, a in enumerate(out_arrs)]
        return [
            {name: full[i][c] for i, name in enumerate(out_names)}
            for c in range(n_cores)
        ]

    b2j.run_bass_via_pjrt = cached_run
    b2j._mjc_patched = True


LAST_TIMES = {}


def kernel(image, depth):
    import time as _time
    image = np.asarray(image, np.float32)
    depth = np.asarray(depth, np.float32)
    try:
        import jax
        import jax.numpy as jnp
        cpu = jax.devices("cpu")[0]
        with jax.default_device(cpu):
            md = np.float32(jax.jit(jnp.mean, backend="cpu")(jax.device_put(depth, cpu)))
    except Exception:
        md = np.float32(np.sum(depth.ravel(), dtype=np.float32) / np.float32(depth.size))

    nc = _build()
    _patch_bass2jax()
    _t0 = _time.time()
    res = run_bass_kernel_spmd(nc, _inputs(image, md), [0])
    LAST_TIMES["A"] = _time.time() - _t0
    LAST_TIMES["B"] = 0.0

    psums = res.results[0]["psums"]
    Sp = np.float32(np.float32(np.sum(psums[:, :3].astype(np.float64))) + np.float32(1e-7))
    po = res.results[0]["pout"].astype(np.float32) * np.float32(2.0 ** 26)
    out = np.empty((3, RES, RES), np.float64)
    for ch in range(3):
        out[ch] = po[ch * 512:ch * 512 + RES, :RES].astype(np.float64)
    out = out / np.float64(Sp)
    return np.clip(out, 0.0, 1.0).astype(np.float32)


# revision 14
# speedup vs baseline: 38.2307x; 1.2662x over previous
"""Trainium2 Bass kernel for nn_MjCambrianOptics (depth-invariant PSF + FFT blur).

Single fused SPMD launch on 8 cores (tunnel-payload optimized).

Every core computes the FULL 3-channel PSF on device (replicated: PE time is
cheap, tunnel bytes are not) via the launch-A algebra E = Winv @ Hs @ W1s @ u2
with bit-exact phases fl(k*r), fl(t1*q) (host IEEE sqrt r ships as input;
Cody-Waite reduction + ACT Sin on device). PSF matmuls run in bf16 (incoherent
rounding noise ~2e-3 ≪ 2e-2 tolerance).

The conv side absorbs the reference's psf flip + fftshift row-perm into
statically reordered twiddles: Fpsf[r,c] = Σ P[i,jj] w^{r(1022-perm_s[i])}
w^{c(766-jj)} (only the 511 nonzero psf cols enter). The image is row-sharded
1/8 per core in bf16; Fimg factors as TwL^T @ (imgT @ WgT) with the global row
offset phase w^{r*128*core} deferred to a per-partition complex scale on Y.
Everything downstream is linear in the image, so per-core fp16 partial outputs
sum on the host. Hermitian row truncation (WLz doubling) keeps r-blocks 0..6.

Per-core payload: img slice bf16 0.79MB + r_supp f32 1.05MB + scal; out
fp16 1.57MB. ~28MB total vs ~250MB for the two-launch baseline.
"""
import numpy as np
import ml_dtypes

import concourse.bacc as bacc
import concourse.mybir as mybir
import concourse.tile as tile
from concourse.bass_utils import run_bass_kernel_spmd

F32 = mybir.dt.float32
F32R = mybir.dt.float32r
BF16 = mybir.dt.bfloat16
F16 = mybir.dt.float16
AF = mybir.ActivationFunctionType
ALU = mybir.AluOpType
AX = mybir.AxisListType

MX = 1023
RES = 511
S0 = 256
NS = 511
SENSOR = 0.01
APERTURE = 0.5
WAVELENGTHS = np.array([610e-9, 530e-9, 470e-9], dtype=np.float32)
P = 1536
NRB = 7                 # freq row blocks 0..6 (896 rows >= 769 Hermitian rows)
NCORE = 8

PI = np.float32(np.pi)
TWO_PI = np.float64(2.0) * np.pi
C_RND = float(np.float32(1.5 * 2.0 ** 23))

KC_S = [(0, 128), (128, 128), (256, 128), (384, 127)]        # 511 rows
KC_M = [(i * 128, 128) for i in range(7)] + [(896, 127)]     # 1023 rows


def _r32c(x):
    """Round ndarray to f32r (12-bit significand), RNE — matches tensor_copy."""
    f = np.ascontiguousarray(x, np.float32)
    b = f.view(np.uint32).astype(np.uint64)
    low = b & 0xFFF
    b2 = b & ~np.uint64(0xFFF)
    up = (low > 0x800) | ((low == 0x800) & (((b2 >> 12) & 1) == 1))
    b2 = b2 + np.where(up, np.uint64(0x1000), np.uint64(0))
    return b2.astype(np.uint32).view(np.float32).reshape(f.shape)


def _bf16(x):
    return np.ascontiguousarray(x, np.float32).astype(ml_dtypes.bfloat16)


def _splitb(x64, keep):
    f = np.float32(x64)
    mask = np.uint32(0xFFFFFFFF ^ ((1 << (24 - keep)) - 1))
    bits = np.uint32(int(f.view(np.uint32)) & int(mask))
    return bits.view(np.float32)


P1 = _splitb(TWO_PI, 11)
P2 = _splitb(TWO_PI - np.float64(P1), 10)
P3 = np.float32(TWO_PI - np.float64(P1) - np.float64(P2))
P1H = np.float32(np.float64(P1) * 256.0)
P2H = np.float32(np.float64(P2) * 256.0)

_CONSTS = {}


def _consts():
    if _CONSTS:
        return _CONSTS
    dx = SENSOR / MX
    Lx = dx * MX
    x1 = np.linspace(-Lx / 2, Lx / 2, MX, dtype=np.float32)
    X1, Y1 = np.meshgrid(x1, x1, indexing="ij")
    fx = np.linspace(-1.0 / (2 * dx), 1.0 / (2 * dx), MX, dtype=np.float32)
    FX, FY = np.meshgrid(fx, fx, indexing="ij")
    ar = (Lx / 2.0) * APERTURE
    A = (np.sqrt(X1 ** 2 + Y1 ** 2) / np.float32(ar + 1e-7) <= 1.0).astype(np.float32)
    lam = WAVELENGTHS
    k_arr = (np.float32(2.0) * np.float32(np.pi) / lam).astype(np.float32)
    q = np.empty((3, MX, MX), np.float32)
    for c in range(3):
        a_ = (lam[c] * FX).astype(np.float32)
        b_ = (lam[c] * FY).astype(np.float32)
        s_ = ((np.float32(1.0) - (a_ * a_).astype(np.float32)).astype(np.float32)
              - (b_ * b_).astype(np.float32)).astype(np.float32)
        q[c] = np.sqrt(s_).astype(np.float32)
    R2 = ((X1 * X1).astype(np.float32) + (Y1 * Y1).astype(np.float32)).astype(np.float32)

    jk = np.arange(MX)
    W = np.exp(-2j * np.pi * np.outer(jk, jk) / MX)
    Winv = np.conj(W) / MX
    perm_s = (jk - MX // 2) % MX
    perm_si = (jk + MX // 2) % MX
    WS_s = W[:, perm_si][:, S0:S0 + NS]        # [1023 x 511]
    qs_sh = q[:, perm_s][:, :, perm_s]         # Hs = exp(i t1 qs_sh)

    jP = np.arange(P)
    Wp = np.exp(-2j * np.pi * np.outer(jP, jP) / P)
    Winvp = np.conj(Wp) / P
    selr = 767 + np.arange(RES)
    WLm = Winvp[selr, :769].copy()
    WLm[:, 1:768] *= 2.0
    WLz = np.zeros((RES, NRB * 128), np.complex128)
    WLz[:, :769] = WLm
    WR = Winvp[selr, :]                        # [511 x 1536]

    kvec = 1022 - perm_s                       # psf row exponent map
    mvec = 766 - np.arange(NS)                 # psf col exponent map

    C = {}
    C["k_arr"] = k_arr
    C["R2supp"] = R2[S0:S0 + NS, S0:S0 + NS]
    amask = np.zeros((512, 512), np.float32)
    amask[:NS, :NS] = A[S0:S0 + NS, S0:S0 + NS]
    C["amask"] = amask
    for c in range(3):
        C[f"qs{c}"] = np.ascontiguousarray(qs_sh[c])

    w1p = np.zeros((NS, 2048), np.float32)
    w1p[:, :MX] = np.real(WS_s).T
    w1p[:, 1024:1024 + MX] = np.imag(WS_s).T
    C["w1s_pack"] = _bf16(w1p)                 # [511 x 2048] bf16 lhsT step1
    wvp = np.zeros((MX, 2048), np.float32)
    wvp[:, :MX] = np.real(Winv)                # symmetric
    wvp[:, 1024:1024 + MX] = np.imag(Winv)
    C["winv_pack"] = _bf16(wvp)

    wgpt = np.zeros((MX, 1792), np.float64)    # [i x (re 896 | im 896)]
    WgP = Wp[:896, :][:, kvec]                 # [896 r x 1023 i]
    wgpt[:, :896] = np.real(WgP).T
    wgpt[:, 896:1792] = np.imag(WgP).T
    C["wgpt_pack"] = _bf16(wgpt)

    wgq = np.zeros((NS, 4608), np.float64)     # [jj x (re|im|imN 1536)]
    WgQ = Wp[mvec, :]                          # [511 jj x 1536 c]
    wgq[:, :1536] = np.real(WgQ)
    wgq[:, 1536:3072] = np.imag(WgQ)
    wgq[:, 3072:4608] = -np.imag(WgQ)
    C["wgq_pack"] = _bf16(wgq)

    wgt = np.zeros((MX, 3072), np.float64)     # [m x (re|im 1536)] w^{cm}
    WgT = Wp[:, :MX].T
    wgt[:, :1536] = np.real(WgT)
    wgt[:, 1536:3072] = np.imag(WgT)
    C["wgt_pack"] = _bf16(wgt)

    twlf = np.zeros((MX, 1792), np.float64)    # [k x (re|im 896)] w^{r k}
    TwF = Wp[:896, :MX].T
    twlf[:, :896] = np.real(TwF)
    twlf[:, 896:1792] = np.imag(TwF)
    C["twlf_pack"] = _bf16(twlf)

    wrt = np.zeros((P, 1536), np.float64)      # [c x (re|im|imN 512)] WR^T
    wrt[:, :RES] = np.real(WR).T
    wrt[:, 512:512 + RES] = np.imag(WR).T
    wrt[:, 1024:1024 + RES] = -np.imag(WR).T
    C["wrt_pack"] = _bf16(wrt)

    wlz = np.zeros((NRB * 128, 1024), np.float64)  # [r x (re 512 | imN 512)]
    wlz[:, :RES] = np.real(WLz).T * (2.0 ** -26)   # fp16-range scale, exact
    wlz[:, 512:512 + RES] = -np.imag(WLz).T * (2.0 ** -26)
    C["wlzt_pack"] = _bf16(wlz)

    C["ident"] = np.eye(128, dtype=np.float32)
    C["Wp"] = Wp
    _CONSTS.update(C)
    return _CONSTS


# ---------------------------------------------------------------------------
_NC = {}


def _build():
    if "nc" in _NC:
        return _NC["nc"]
    nc = bacc.Bacc("TRN2", target_bir_lowering=False, debug=False)
    C = _consts()
    ins = {}
    for nm in ["wgpt_pack", "wgq_pack", "wgt_pack", "twlf_pack", "wrt_pack",
               "wlzt_pack", "w1s_pack", "winv_pack"]:
        ins[nm] = nc.inline_tensor(C[nm], nm).ap()
    for c in range(3):
        ins[f"qs{c}"] = nc.inline_tensor(C[f"qs{c}"], f"qs{c}").ap()
    ins["amask"] = nc.inline_tensor(C["amask"], "amask").ap()
    ins["ident"] = nc.inline_tensor(C["ident"], "ident").ap()
    blob = nc.dram_tensor("blob", [3712, 512], mybir.dt.uint32,
                          kind="ExternalInput").ap()
    ins["img"] = blob.bitcast(BF16)          # [3712, 1024]; rows 0:3072
    ins["rs"] = blob.bitcast(F32)            # [3712, 512]; rows 3072:3584
    ins["sc"] = blob.bitcast(F32)            # rows 3584:3712, cols 0:32
    pout = nc.dram_tensor("pout", [1664, 512], F16, kind="ExternalOutput").ap()

    with tile.TileContext(nc) as tc:
        with (
            tc.tile_pool(name="cst", bufs=1) as cp,
            tc.tile_pool(name="acc", bufs=1) as ao,
        ):
            scal = cp.tile([128, 32], F32, tag="scal")
            nc.sync.dma_start(scal[:], ins["sc"][3584:3712, :32])
            ident = cp.tile([128, 128], F32, tag="ident")
            nc.sync.dma_start(ident[:], ins["ident"][:])
            lrs = cp.tile([128, 4 * 512], F32, tag="lrs")
            lmsk = cp.tile([128, 4 * 512], F32, tag="lmsk")
            for ci, (r0, rn) in enumerate(KC_S):
                nc.sync.dma_start(lrs[:rn, ci * 512:(ci + 1) * 512],
                                  ins["rs"][3072 + r0:3072 + r0 + rn, :])
                nc.sync.dma_start(lmsk[:rn, ci * 512:(ci + 1) * 512],
                                  ins["amask"][r0:r0 + rn, :])
            oacc = ao.tile([128, 3 * 2048], F32, tag="oacc")
            nc.vector.memset(oacc[:], 0.0)

            def trig_pair(dst_cos, dst_sin, base_ap, t_col, rows, w,
                          tg, mask_ap=None):
                th = tg.tile([128, MX], F32, tag="th")
                nc.vector.tensor_scalar_mul(th[:rows, :w], base_ap,
                                            scal[:rows, t_col:t_col + 1])
                f = tg.tile([128, MX], F32, tag="f")
                nc.vector.tensor_scalar(f[:rows, :w], th[:rows, :w],
                                        float(np.float32(1.0 / TWO_PI)), C_RND,
                                        ALU.mult, ALU.add)
                nc.vector.tensor_scalar_sub(f[:rows, :w], f[:rows, :w], C_RND)
                g = tg.tile([128, MX], F32, tag="g")
                nc.vector.tensor_scalar(g[:rows, :w], f[:rows, :w],
                                        float(np.float32(1.0 / 256.0)), C_RND,
                                        ALU.mult, ALU.add)
                nc.vector.tensor_scalar_sub(g[:rows, :w], g[:rows, :w], C_RND)
                nl = tg.tile([128, MX], F32, tag="nl")
                nc.vector.cody_waite_cascade(nl[:rows, :w], f[:rows, :w],
                                             g[:rows, :w], 256.0, 0.0, 0.0)
                y = tg.tile([128, MX], F32, tag="y")
                nc.vector.cody_waite_cascade(y[:rows, :w], th[:rows, :w],
                                             g[:rows, :w], float(P1H), float(P2H), 0.0)
                nc.vector.cody_waite_cascade(y[:rows, :w], y[:rows, :w],
                                             nl[:rows, :w], float(P1), float(P2), 0.0)
                nc.vector.cody_waite_cascade(y[:rows, :w], y[:rows, :w],
                                             f[:rows, :w], float(P3), 0.0, 0.0)
                yw = tg.tile([128, MX], F32, tag="yw")
                nc.vector.add_range_wrap(yw[:rows, :w], y[:rows, :w], 0.0,
                                         float(PI), float(np.float32(2 * np.pi)))
                yc = tg.tile([128, MX], F32, tag="yc")
                nc.vector.add_range_wrap(yc[:rows, :w], y[:rows, :w],
                                         float(np.float32(PI / 2)), float(PI),
                                         float(np.float32(2 * np.pi)))
                if mask_ap is None:
                    nc.scalar.activation(dst_sin, yw[:rows, :w], AF.Sin)
                    nc.scalar.activation(dst_cos, yc[:rows, :w], AF.Sin)
                else:
                    sn = tg.tile([128, 256], F32, tag="sn")
                    cn = tg.tile([128, 256], F32, tag="cn")
                    nc.scalar.activation(sn[:rows, :w], yw[:rows, :w], AF.Sin)
                    nc.scalar.activation(cn[:rows, :w], yc[:rows, :w], AF.Sin)
                    nc.vector.tensor_tensor(dst_sin, sn[:rows, :w], mask_ap, ALU.mult)
                    nc.vector.tensor_tensor(dst_cos, cn[:rows, :w], mask_ap, ALU.mult)

            for ch in range(3):
                with tc.tile_pool(name="chan", bufs=1) as chp:
                    ps_t = chp.tile([128, 8 * 512], BF16, tag="ps_t")
                    nc.vector.memset(ps_t[:], 0.0)

                    # ======== PSF phase: E = Winv @ Hs @ W1s @ u2 ========
                    with (
                        tc.tile_pool(name="psfp", bufs=1) as pf,
                        tc.tile_pool(name="trg", bufs=1) as tg,
                        tc.tile_pool(name="wbf", bufs=2) as wb,
                        tc.tile_pool(name="psA", bufs=2, space="PSUM") as pp,
                    ):
                        # Hs trig (resident bf16, 8 chunks)
                        hs = {}
                        for ci, (r0, rn) in enumerate(KC_M):
                            qsl = tg.tile([128, MX], F32, tag="qsl")
                            nc.sync.dma_start(qsl[:rn], ins[f"qs{ch}"][r0:r0 + rn, :])
                            hre = pf.tile([128, MX], BF16, tag=f"hre{ci}")
                            him = pf.tile([128, MX], BF16, tag=f"him{ci}")
                            trig_pair(hre[:rn], him[:rn], qsl[:rn], 3 + ch,
                                      rn, MX, tg)
                            hs[ci] = (hre, him)

                        # u2 trig: per chunk [h0re 256|h0im 256|h1re|h1im]
                        u2_m = pf.tile([128, 4 * 1024], BF16, tag="u2m")
                        u2_n = pf.tile([128, 4 * 512], BF16, tag="u2n")
                        nc.vector.memset(u2_m[:], 0.0)
                        for ci, (r0, rn) in enumerate(KC_S):
                            for h in range(2):
                                o = ci * 1024 + h * 512
                                base = lrs[:rn, ci * 512 + h * 256:
                                           ci * 512 + h * 256 + 256]
                                mk = lmsk[:rn, ci * 512 + h * 256:
                                          ci * 512 + h * 256 + 256]
                                trig_pair(u2_m[:rn, o:o + 256],
                                          u2_m[:rn, o + 256:o + 512],
                                          base, ch, rn, 256, tg, mask_ap=mk)
                            nc.vector.tensor_scalar_mul(
                                u2_n[:rn, ci * 512:ci * 512 + 256],
                                u2_m[:rn, ci * 1024 + 256:ci * 1024 + 512], -1.0)
                            nc.vector.tensor_scalar_mul(
                                u2_n[:rn, ci * 512 + 256:ci * 512 + 512],
                                u2_m[:rn, ci * 1024 + 768:ci * 1024 + 1024], -1.0)

                        def cmm(acc, lre, lim, m_full, m_re, m_im_neg, first, last):
                            nc.tensor.matmul(acc[:, 0:512], lre, m_full,
                                             start=first, stop=False)
                            nc.tensor.matmul(acc[:, 0:256], lim, m_im_neg,
                                             start=False, stop=False)
                            nc.tensor.matmul(acc[:, 256:512], lim, m_re,
                                             start=False, stop=last)

                        for h in range(2):
                            # ---- step1: X1 = W1s @ u2(half) ----
                            x1_m = pf.tile([128, 8 * 512], BF16, tag="x1m")
                            x1_n = pf.tile([128, 8 * 256], BF16, tag="x1n")
                            for mi, (m0, mn) in enumerate(KC_M):
                                acc = pp.tile([128, 512], F32, tag="accA")
                                for ci, (r0, rn) in enumerate(KC_S):
                                    lw = wb.tile([128, 2048], BF16, tag="wbf")
                                    nc.sync.dma_start(
                                        lw[:rn, :mn],
                                        ins["w1s_pack"][r0:r0 + rn, m0:m0 + mn])
                                    nc.sync.dma_start(
                                        lw[:rn, 1024:1024 + mn],
                                        ins["w1s_pack"][r0:r0 + rn,
                                                        1024 + m0:1024 + m0 + mn])
                                    o = ci * 1024 + h * 512
                                    cmm(acc[:mn], lw[:rn, :mn],
                                        lw[:rn, 1024:1024 + mn],
                                        u2_m[:rn, o:o + 512],
                                        u2_m[:rn, o:o + 256],
                                        u2_n[:rn, ci * 512 + h * 256:
                                             ci * 512 + h * 256 + 256],
                                        ci == 0, ci == 3)
                                o = mi * 512
                                nc.vector.tensor_copy(x1_m[:mn, o:o + 512], acc[:mn])
                                nc.vector.tensor_scalar_mul(
                                    x1_n[:mn, mi * 256:(mi + 1) * 256],
                                    acc[:mn, 256:512], -1.0)

                            # ---- step2: X2 = Hs @ X1 ----
                            x2_m = pf.tile([128, 8 * 512], BF16, tag="x2m")
                            x2_n = pf.tile([128, 8 * 256], BF16, tag="x2n")
                            for mi, (m0, mn) in enumerate(KC_M):
                                acc = pp.tile([128, 512], F32, tag="accB")
                                for ci, (r0, rn) in enumerate(KC_M):
                                    hre, him = hs[ci]
                                    o = ci * 512
                                    cmm(acc[:mn], hre[:rn, m0:m0 + mn],
                                        him[:rn, m0:m0 + mn],
                                        x1_m[:rn, o:o + 512],
                                        x1_m[:rn, o:o + 256],
                                        x1_n[:rn, ci * 256:(ci + 1) * 256],
                                        ci == 0, ci == 7)
                                o = mi * 512
                                nc.vector.tensor_copy(x2_m[:mn, o:o + 512], acc[:mn])
                                nc.vector.tensor_scalar_mul(
                                    x2_n[:mn, mi * 256:(mi + 1) * 256],
                                    acc[:mn, 256:512], -1.0)

                            # ---- step3: E = Winv @ X2; ps_t += |E|^2 ----
                            for mi, (m0, mn) in enumerate(KC_M):
                                acc = pp.tile([128, 512], F32, tag="accC")
                                for ci, (r0, rn) in enumerate(KC_M):
                                    lw = wb.tile([128, 2048], BF16, tag="wbf")
                                    nc.sync.dma_start(
                                        lw[:rn, :mn],
                                        ins["winv_pack"][r0:r0 + rn, m0:m0 + mn])
                                    nc.sync.dma_start(
                                        lw[:rn, 1024:1024 + mn],
                                        ins["winv_pack"][r0:r0 + rn,
                                                         1024 + m0:1024 + m0 + mn])
                                    o = ci * 512
                                    cmm(acc[:mn], lw[:rn, :mn],
                                        lw[:rn, 1024:1024 + mn],
                                        x2_m[:rn, o:o + 512],
                                        x2_m[:rn, o:o + 256],
                                        x2_n[:rn, ci * 256:(ci + 1) * 256],
                                        ci == 0, ci == 7)
                                e_sb = tg.tile([128, 512], F32, tag="esb")
                                nc.vector.tensor_copy(e_sb[:mn], acc[:mn])
                                sq = tg.tile([128, 256], F32, tag="sq")
                                nc.vector.tensor_tensor(sq[:mn], e_sb[:mn, 0:256],
                                                        e_sb[:mn, 0:256], ALU.mult)
                                sq2 = tg.tile([128, 256], F32, tag="sq2")
                                nc.vector.tensor_tensor(sq2[:mn], e_sb[:mn, 256:512],
                                                        e_sb[:mn, 256:512], ALU.mult)
                                nc.vector.tensor_tensor(
                                    ps_t[:mn, mi * 512 + h * 256:
                                         mi * 512 + h * 256 + 256],
                                    sq[:mn], sq2[:mn], ALU.add)

                    # psf channel sum -> psums[:, ch]
                    rsum = cp.tile([128, 4], F32, tag="rsum")
                    rtmp = cp.tile([128, 8 * 512], F32, tag="rtmp")
                    nc.vector.tensor_copy(rtmp[:], ps_t[:])
                    nc.vector.tensor_reduce(rsum[:, ch:ch + 1],
                                            rtmp[:], AX.X, ALU.add)
                    rs16 = cp.tile([128, 4], F16, tag="rs16")
                    nc.vector.tensor_scalar_mul(rs16[:, ch:ch + 1],
                                                rsum[:, ch:ch + 1],
                                                float(2.0 ** -26))
                    nc.sync.dma_start(pout[1536:1664, ch:ch + 1],
                                      rs16[:, ch:ch + 1])

                    # ======== CONV phase ========
                    with (
                        tc.tile_pool(name="convp", bufs=1) as cv,
                        tc.tile_pool(name="str", bufs=2) as stp,
                    ):
                        # ---- T[m, r] = sum_k img[k, m] w^{rk} (col-DFT) ----
                        imt = [cv.tile([128, 1024], BF16, tag=f"imt{b}",
                                       name=f"imt{b}") for b in range(8)]
                        for b, (k0, kn) in enumerate(KC_M):
                            nc.sync.dma_start(
                                imt[b][:kn],
                                ins["img"][ch * 1024 + k0:ch * 1024 + k0 + kn, :])
                        tpi = [cv.tile([128, 2688], BF16, tag=f"tpi{mc}",
                                       name=f"tpi{mc}") for mc in range(8)]
                        with tc.tile_pool(name="psI", bufs=2, space="PSUM") as pg:
                            for rs0, rw in ((0, 512), (512, 384)):
                                for mc, (m0, mn) in enumerate(KC_M):
                                    a_re = pg.tile([128, 512], F32, tag="ti_re")
                                    a_im = pg.tile([128, 512], F32, tag="ti_im")
                                    for kc, (k0, kn) in enumerate(KC_M):
                                        sw = stp.tile([128, 4608], BF16, tag="str")
                                        nc.sync.dma_start(
                                            sw[:kn, :rw],
                                            ins["twlf_pack"][k0:k0 + kn, rs0:rs0 + rw])
                                        nc.sync.dma_start(
                                            sw[:kn, 1024:1024 + rw],
                                            ins["twlf_pack"][k0:k0 + kn,
                                                             896 + rs0:896 + rs0 + rw])
                                        lhs = imt[kc][:kn, m0:m0 + mn]
                                        nc.tensor.matmul(a_re[:mn, :rw], lhs,
                                                         sw[:kn, :rw],
                                                         start=(kc == 0), stop=(kc == 7))
                                        nc.tensor.matmul(a_im[:mn, :rw], lhs,
                                                         sw[:kn, 1024:1024 + rw],
                                                         start=(kc == 0), stop=(kc == 7))
                                    nc.vector.tensor_copy(
                                        tpi[mc][:mn, rs0:rs0 + rw], a_re[:mn, :rw])
                                    nc.vector.tensor_copy(
                                        tpi[mc][:mn, 896 + rs0:896 + rs0 + rw],
                                        a_im[:mn, :rw])
                                    nc.vector.tensor_scalar_mul(
                                        tpi[mc][:mn, 1792 + rs0:1792 + rs0 + rw],
                                        a_im[:mn, :rw], -1.0)

                        # ---- tmpT_p[jj, r] = sum_i P[i,jj] WgP[r,i] ----
                        tpp = [cv.tile([128, 1792], BF16, tag=f"tpp{j}", name=f"tpp{j}")
                               for j in range(4)]
                        with tc.tile_pool(name="psT", bufs=2, space="PSUM") as pg:
                            for rs0, rw in ((0, 512), (512, 384)):
                                for jc, (j0, jn) in enumerate(KC_S):
                                    a_re = pg.tile([128, 512], F32, tag="tp_re")
                                    a_im = pg.tile([128, 512], F32, tag="tp_im")
                                    for ic, (i0, icn) in enumerate(KC_M):
                                        sw = stp.tile([128, 4608], BF16, tag="str")
                                        nc.sync.dma_start(
                                            sw[:icn, :rw],
                                            ins["wgpt_pack"][i0:i0 + icn, rs0:rs0 + rw])
                                        nc.sync.dma_start(
                                            sw[:icn, 1024:1024 + rw],
                                            ins["wgpt_pack"][i0:i0 + icn,
                                                             896 + rs0:896 + rs0 + rw])
                                        lhs = ps_t[:icn, ic * 512 + j0:ic * 512 + j0 + jn]
                                        nc.tensor.matmul(a_re[:jn, :rw], lhs,
                                                         sw[:icn, :rw],
                                                         start=(ic == 0), stop=(ic == 7))
                                        nc.tensor.matmul(a_im[:jn, :rw], lhs,
                                                         sw[:icn, 1024:1024 + rw],
                                                         start=(ic == 0), stop=(ic == 7))
                                    nc.vector.tensor_copy(
                                        tpp[jc][:jn, rs0:rs0 + rw], a_re[:jn, :rw])
                                    nc.vector.tensor_copy(
                                        tpp[jc][:jn, 896 + rs0:896 + rs0 + rw],
                                        a_im[:jn, :rw])

                        # ---- per freq-row-block pipeline ----
                        with tc.tile_pool(name="psC", bufs=1, space="PSUM") as pq:
                            for rb in range(NRB):
                                dT_re = cv.tile([128, 1536], BF16, tag="dTre")
                                dT_im = cv.tile([128, 1536], BF16, tag="dTim")
                                for cs in range(3):
                                    # Fpsf accum
                                    p_re = pq.tile([128, 512], F32, tag="p_re")
                                    p_im = pq.tile([128, 512], F32, tag="p_im")
                                    for jc, (j0, jn) in enumerate(KC_S):
                                        sw = stp.tile([128, 4608], BF16, tag="str")
                                        nc.sync.dma_start(
                                            sw[:jn], ins["wgq_pack"][j0:j0 + jn, :])
                                        t_re = tpp[jc][:jn, rb * 128:rb * 128 + 128]
                                        t_im = tpp[jc][:jn,
                                                       896 + rb * 128:896 + rb * 128 + 128]
                                        q_re = sw[:jn, cs * 512:cs * 512 + 512]
                                        q_im = sw[:jn, 1536 + cs * 512:1536 + cs * 512 + 512]
                                        q_imN = sw[:jn, 3072 + cs * 512:3072 + cs * 512 + 512]
                                        nc.tensor.matmul(p_re[:], t_re, q_re,
                                                         start=(jc == 0), stop=False)
                                        nc.tensor.matmul(p_re[:], t_im, q_imN,
                                                         start=False, stop=(jc == 3))
                                        nc.tensor.matmul(p_im[:], t_re, q_im,
                                                         start=(jc == 0), stop=False)
                                        nc.tensor.matmul(p_im[:], t_im, q_re,
                                                         start=False, stop=(jc == 3))
                                    # Fimg accum: sum_m T[m,r] WgT[m,c]
                                    i_re = pq.tile([128, 512], F32, tag="i_re")
                                    i_im = pq.tile([128, 512], F32, tag="i_im")
                                    for mc, (m0, mn) in enumerate(KC_M):
                                        sw = stp.tile([128, 4608], BF16, tag="str")
                                        nc.sync.dma_start(
                                            sw[:mn, :512],
                                            ins["wgt_pack"][m0:m0 + mn,
                                                            cs * 512:cs * 512 + 512])
                                        nc.sync.dma_start(
                                            sw[:mn, 1024:1536],
                                            ins["wgt_pack"][m0:m0 + mn,
                                                            1536 + cs * 512:1536 + cs * 512 + 512])
                                        t_re = tpi[mc][:mn, rb * 128:rb * 128 + 128]
                                        t_im = tpi[mc][:mn,
                                                       896 + rb * 128:896 + rb * 128 + 128]
                                        t_imN = tpi[mc][:mn,
                                                        1792 + rb * 128:1792 + rb * 128 + 128]
                                        w_re = sw[:mn, :512]
                                        w_im = sw[:mn, 1024:1536]
                                        nc.tensor.matmul(i_re[:], t_re, w_re,
                                                         start=(mc == 0), stop=False)
                                        nc.tensor.matmul(i_re[:], t_imN, w_im,
                                                         start=False, stop=(mc == 7))
                                        nc.tensor.matmul(i_im[:], t_re, w_im,
                                                         start=(mc == 0), stop=False)
                                        nc.tensor.matmul(i_im[:], t_im, w_re,
                                                         start=False, stop=(mc == 7))
                                    # D = Fimg .* Fpsf  (per 512-col seg)
                                    fir = cv.tile([128, 512], F32, tag="fir")
                                    fii = cv.tile([128, 512], F32, tag="fii")
                                    nc.vector.tensor_copy(fir[:], i_re[:])
                                    nc.vector.tensor_copy(fii[:], i_im[:])
                                    t1_ = cv.tile([128, 512], F32, tag="t1")
                                    t2_ = cv.tile([128, 512], F32, tag="t2")
                                    d_re = cv.tile([128, 512], F32, tag="dre")
                                    d_im = cv.tile([128, 512], F32, tag="dim")
                                    nc.vector.tensor_tensor(t1_[:], fir[:], p_re[:],
                                                            ALU.mult)
                                    nc.vector.tensor_tensor(t2_[:], fii[:], p_im[:],
                                                            ALU.mult)
                                    nc.vector.tensor_tensor(d_re[:], t1_[:], t2_[:],
                                                            ALU.subtract)
                                    nc.vector.tensor_tensor(t1_[:], fir[:], p_im[:],
                                                            ALU.mult)
                                    nc.vector.tensor_tensor(t2_[:], fii[:], p_re[:],
                                                            ALU.mult)
                                    nc.vector.tensor_tensor(d_im[:], t1_[:], t2_[:],
                                                            ALU.add)
                                    # transpose D seg -> dT chunks
                                    for b in range(4):
                                        ci = cs * 4 + b
                                        for pl, dst in ((d_re, dT_re), (d_im, dT_im)):
                                            ptr = pq.tile([128, 128], F32, tag="ptrD")
                                            nc.tensor.transpose(
                                                ptr[:], pl[:, b * 128:(b + 1) * 128],
                                                ident[:])
                                            nc.vector.tensor_copy(
                                                dst[:, ci * 128:(ci + 1) * 128], ptr[:])
                                # ---- s1: Y = D^T-contract @ WR^T ----
                                y_re = pq.tile([128, 512], F32, tag="y_re")
                                y_im = pq.tile([128, 512], F32, tag="y_im")
                                for ci in range(12):
                                    sw = stp.tile([128, 4608], BF16, tag="str")
                                    nc.sync.dma_start(
                                        sw[:128, :1536],
                                        ins["wrt_pack"][ci * 128:(ci + 1) * 128, :])
                                    dre_c = dT_re[:, ci * 128:(ci + 1) * 128]
                                    dim_c = dT_im[:, ci * 128:(ci + 1) * 128]
                                    w_re = sw[:128, 0:512]
                                    w_im = sw[:128, 512:1024]
                                    w_imN = sw[:128, 1024:1536]
                                    nc.tensor.matmul(y_re[:], dre_c, w_re,
                                                     start=(ci == 0), stop=False)
                                    nc.tensor.matmul(y_re[:], dim_c, w_imN,
                                                     start=False, stop=(ci == 11))
                                    nc.tensor.matmul(y_im[:], dre_c, w_im,
                                                     start=(ci == 0), stop=False)
                                    nc.tensor.matmul(y_im[:], dim_c, w_re,
                                                     start=False, stop=(ci == 11))
                                yp_re = cv.tile([128, 512], BF16, tag="ypre")
                                yp_im = cv.tile([128, 512], BF16, tag="ypim")
                                nc.vector.tensor_copy(yp_re[:], y_re[:])
                                nc.vector.tensor_copy(yp_im[:], y_im[:])
                                # ---- WL accumulation into oacc ----
                                swl = stp.tile([128, 4608], BF16, tag="str")
                                nc.sync.dma_start(
                                    swl[:128, :1024],
                                    ins["wlzt_pack"][rb * 128:(rb + 1) * 128, :])
                                for xt in range(4):
                                    accO = pq.tile([128, 512], F32, tag="accO")
                                    nc.tensor.matmul(
                                        accO[:], swl[:128, xt * 128:xt * 128 + 128],
                                        yp_re[:], start=True, stop=False)
                                    nc.tensor.matmul(
                                        accO[:], swl[:128, 512 + xt * 128:512 + xt * 128 + 128],
                                        yp_im[:], start=False, stop=True)
                                    osl = oacc[:, ch * 2048 + xt * 512:
                                               ch * 2048 + (xt + 1) * 512]
                                    nc.vector.tensor_tensor(osl, osl, accO[:], ALU.add)

            # ---- emit partial outputs as fp16 ----
            for ch in range(3):
                for xt in range(4):
                    oo16 = cp.tile([128, 512], F16, tag="oo16")
                    nc.vector.tensor_copy(
                        oo16[:], oacc[:, ch * 2048 + xt * 512:ch * 2048 + (xt + 1) * 512])
                    nc.sync.dma_start(
                        pout[ch * 512 + xt * 128:ch * 512 + (xt + 1) * 128, :], oo16[:])

    nc.compile()
    _NC["nc"] = nc
    return nc


def _inputs(image, md):
    C = _consts()
    k_arr = C["k_arr"]
    m2 = np.float32(md * md)
    r_supp = np.sqrt((C["R2supp"] + m2).astype(np.float32)).astype(np.float32)
    rs = np.zeros((512, 512), np.float32)
    rs[:NS, :NS] = r_supp
    img = np.zeros((3072, 1024), ml_dtypes.bfloat16)
    for ch in range(3):
        img[ch * 1024:ch * 1024 + MX, :MX] = _bf16(image[ch])
    sc = np.zeros((128, 32), np.float32)
    for ch in range(3):
        sc[:, ch] = k_arr[ch]
        sc[:, 3 + ch] = np.float32(k_arr[ch] * md)
    blob = np.zeros((3712, 512), np.uint32)
    blob[0:3072] = img.view(np.uint16).astype(np.uint32).reshape(3072, 512, 2)[
        :, :, 0] | (img.view(np.uint16).astype(np.uint32).reshape(3072, 512, 2)[
            :, :, 1] << np.uint32(16))
    blob[3072:3584] = rs.view(np.uint32)
    blob[3584:3712, :32] = sc.view(np.uint32)
    return [{"blob": blob}]


_PJIT_CACHE = {}


def _patch_bass2jax():
    """Cache the pjit callable per-(nc, n_cores) across calls.

    Upstream run_bass_via_pjrt builds a fresh jit closure every call, so each
    launch re-traces + re-compiles (~8s client-side for this NEFF) even though
    the executable is unchanged. This patch keeps upstream semantics (same
    lowering, same shard_map layout, donation) but reuses the compiled
    executable, and converts each output with a single np.asarray.
    """
    from concourse import bass2jax as b2j
    if getattr(b2j, "_mjc_patched", False):
        return
    import jax
    from jax.sharding import Mesh, PartitionSpec
    from jax.experimental.shard_map import shard_map

    def cached_run(nc, in_maps, n_cores):
        import concourse.mybir as _mybir
        b2j.install_neuronx_cc_hook()
        key = (id(nc), n_cores)
        ent = _PJIT_CACHE.get(key)
        if ent is None:
            partition_name = (nc.partition_id_tensor.name
                              if nc.partition_id_tensor else None)
            in_names, out_names, out_avals, zero_shapes = [], [], [], []
            for alloc in nc.m.functions[0].allocations:
                if not isinstance(alloc, _mybir.MemoryLocationSet):
                    continue
                name = alloc.memorylocations[0].name
                if alloc.kind == "ExternalInput":
                    if name != partition_name:
                        in_names.append(name)
                elif alloc.kind == "ExternalOutput":
                    out_names.append(name)
                    shape = tuple(alloc.tensor_shape)
                    dtype = _mybir.dt.np(alloc.dtype)
                    out_avals.append(jax.core.ShapedArray(shape, dtype))
                    zero_shapes.append((shape, dtype))
            n_params = len(in_names)
            n_outs = len(out_avals)
            all_names = list(in_names) + list(out_names)
            if partition_name is not None:
                all_names.append(partition_name)
            donate = tuple(range(n_params, n_params + n_outs))

            def _body(*args):
                operands = list(args)
                if partition_name is not None:
                    operands.append(b2j.partition_id_tensor())
                outs = b2j._bass_exec_p.bind(
                    *operands,
                    out_avals=tuple(out_avals),
                    in_names=tuple(all_names),
                    out_names=tuple(out_names),
                    lowering_input_output_aliases=(),
                    sim_require_finite=True,
                    sim_require_nnan=True,
                    nc=nc,
                )
                return tuple(outs)

            devices = jax.devices()[:n_cores]
            mesh = Mesh(np.asarray(devices), ("core",))
            in_specs = (PartitionSpec("core"),) * (n_params + n_outs)
            out_specs = (PartitionSpec("core"),) * n_outs
            sharded = jax.jit(
                shard_map(_body, mesh=mesh, in_specs=in_specs,
                          out_specs=out_specs, check_rep=False),
                donate_argnums=donate, keep_unused=True)
            ent = (in_names, out_names, out_avals, zero_shapes, sharded)
            _PJIT_CACHE[key] = ent
        in_names, out_names, out_avals, zero_shapes, sharded = ent
        concat_in = [
            np.concatenate([np.asarray(m[name]) for m in in_maps], axis=0)
            for name in in_names
        ]
        concat_zeros = [
            np.zeros((n_cores * s[0], *s[1:]), d) for (s, d) in zero_shapes
        ]
        out_arrs = sharded(*concat_in, *concat_zeros)
        full = [np.asarray(a).reshape(n_cores, *out_avals[i].shape)
                for i, a in enumerate(out_arrs)]
        return [
            {name: full[i][c] for i, name in enumerate(out_names)}
            for c in range(n_cores)
        ]

    b2j.run_bass_via_pjrt = cached_run
    b2j._mjc_patched = True


LAST_TIMES = {}


def kernel(image, depth):
    import time as _time
    image = np.asarray(image, np.float32)
    depth = np.asarray(depth, np.float32)
    try:
        import jax
        import jax.numpy as jnp
        cpu = jax.devices("cpu")[0]
        with jax.default_device(cpu):
            md = np.float32(jax.jit(jnp.mean, backend="cpu")(jax.device_put(depth, cpu)))
    except Exception:
        md = np.float32(np.sum(depth.ravel(), dtype=np.float32) / np.float32(depth.size))

    nc = _build()
    _patch_bass2jax()
    _t0 = _time.time()
    res = run_bass_kernel_spmd(nc, _inputs(image, md), [0])
    LAST_TIMES["A"] = _time.time() - _t0
    LAST_TIMES["B"] = 0.0

    pofull = res.results[0]["pout"]
    psums = pofull[1536:1664, :3].astype(np.float64) * (2.0 ** 26)
    Sp = np.float32(np.float32(np.sum(psums)) + np.float32(1e-7))
    po = pofull[:1536].astype(np.float32) * np.float32(2.0 ** 26)
    out = np.empty((3, RES, RES), np.float64)
    for ch in range(3):
        out[ch] = po[ch * 512:ch * 512 + RES, :RES].astype(np.float64)
    out = out / np.float64(Sp)
    return np.clip(out, 0.0, 1.0).astype(np.float32)
